# revision 14
# baseline (speedup 1.0000x reference)
"""LSTM-like policy net on 8 Trainium2 cores, tensor-parallel over the gate dim.

Per-core shard m owns gate rows [256m, 256m+256) of each gate (i,f,g,o),
h elements [256m, 256m+256), and fc rows [512m, 512m+512).
Per step: local W_hh @ h matvec (128 accumulating matmuls), nonlinearities,
AllGather of the 256-element h shard. x-path (embeddings @ W_ih) is
precomputed for all 512 steps in one parallel pass.
"""

import os
import sys

import numpy as np

if "/opt/trn_rl_repo" not in sys.path:
    sys.path.insert(0, "/opt/trn_rl_repo")

T = 512          # sequence length
D = 2048         # input feature dim (2 x 1024 embeddings)
H = 2048         # hidden dim
L = 1024         # local gate rows per core (4 gates x 256)
V = 4096         # fc output dim
M = 8            # cores
NK = 16          # 128-chunks over D/H
NJ = 8           # 128-chunks over L

_CACHE = {}


def _h_perm():
    # h_all flat index u = 256*q + 2*p + jl  ->  h element 256*q + 128*jl + p
    u = np.arange(H)
    q, r = u // 256, u % 256
    return 256 * q + 128 * (r % 2) + r // 2


def _contract_perm():
    # whh_sb row v = 128*k + kk multiplies rhs element h_all[16*kk + k]
    v = np.arange(H)
    k, kk = v // 128, v % 128
    return _h_perm()[16 * kk + k]


def _prep_in_maps(inputs):
    gz = np.ascontiguousarray(np.asarray(inputs["guesses"]).astype(np.int32).ravel())
    fb = np.ascontiguousarray(np.asarray(inputs["feedbacks"]).astype(np.int32).ravel())
    ge = np.asarray(inputs["guess_embed"], dtype=np.float32)
    fe = np.asarray(inputs["feedback_embed"], dtype=np.float32)
    W_ih = np.asarray(inputs["W_ih"], dtype=np.float32)
    W_hh = np.asarray(inputs["W_hh"], dtype=np.float32)
    bias = (np.asarray(inputs["b_ih"], dtype=np.float32)
            + np.asarray(inputs["b_hh"], dtype=np.float32))
    W_fc = np.asarray(inputs["W_fc"], dtype=np.float32)
    b_fc = np.asarray(inputs["b_fc"], dtype=np.float32)

    cperm = _contract_perm()
    in_maps = []
    for m in range(M):
        # local gate row r = 128*jj + p  ->  global row 2048*(jj//2) + 256*m + 128*(jj%2) + p
        jj = np.arange(NJ)
        rows = (2048 * (jj // 2)[:, None] + 256 * m + 128 * (jj % 2)[:, None]
                + np.arange(128)[None, :]).ravel()
        Wih_sh = W_ih[rows]            # [1024, 2048]
        Whh_sh = W_hh[rows]            # [1024, 2048]
        b_sh = np.ascontiguousarray(bias[rows])

        # x-space feature permutation: own embedding half rolled so this
        # core's 256 c_in features land at positions 0:256
        own = ge if m < 4 else fe
        oth = fe if m < 4 else ge
        own_base = 0 if m < 4 else 1024
        roll = (np.arange(1024) + 256 * (m % 4)) % 1024
        perm = np.concatenate([own_base + roll, (1024 - own_base) + np.arange(1024)])

        in_maps.append({
            "idx_a": gz if m < 4 else fb,
            "idx_b": fb if m < 4 else gz,
            "tab_a": np.ascontiguousarray(own[:, roll]),
            "tab_b": np.ascontiguousarray(oth),
            "wih_t": np.ascontiguousarray(Wih_sh[:, perm].T),      # [2048, 1024]
            "whh_t": np.ascontiguousarray(Whh_sh[:, cperm].T),     # [2048, 1024]
            # reordered so a contiguous [[8,128],[1,8]] load puts b_sh[128j+p] at (p,j)
            "bias": np.ascontiguousarray(b_sh.reshape(8, 128).T.ravel()),
            "wfc_t": np.ascontiguousarray(W_fc[512 * m:512 * m + 512][:, cperm].T),
            "bfc": np.ascontiguousarray(
                b_fc[512 * m:512 * m + 512].reshape(4, 128).T.ravel()),
        })
    return in_maps


def _build():
    from concourse import bass, mybir

    f32 = mybir.dt.float32
    i32 = mybir.dt.int32
    Sig = mybir.ActivationFunctionType.Sigmoid
    Tnh = mybir.ActivationFunctionType.Tanh
    ExpF = mybir.ActivationFunctionType.Exp
    Cpy = mybir.ActivationFunctionType.Copy
    AP = bass.AP

    nc = bass.Bass(target_bir_lowering=False, debug=False)

    idx_a = nc.declare_dram_parameter("idx_a", [T], i32, isOutput=False)
    idx_b = nc.declare_dram_parameter("idx_b", [T], i32, isOutput=False)
    tab_a = nc.declare_dram_parameter("tab_a", [4097, 1024], f32, isOutput=False)
    tab_b = nc.declare_dram_parameter("tab_b", [4097, 1024], f32, isOutput=False)
    wih_t = nc.declare_dram_parameter("wih_t", [D, L], f32, isOutput=False)
    whh_t = nc.declare_dram_parameter("whh_t", [H, L], f32, isOutput=False)
    bias_d = nc.declare_dram_parameter("bias", [L], f32, isOutput=False)
    wfc_t = nc.declare_dram_parameter("wfc_t", [H, 512], f32, isOutput=False)
    bfc_d = nc.declare_dram_parameter("bfc", [512], f32, isOutput=False)
    out_ext = nc.declare_dram_parameter("out", [V], f32, isOutput=True)

    h_loc = nc.dram_tensor("h_loc", [256], f32)
    h_all = nc.dram_tensor("h_all", [H], f32, addr_space="Shared")
    e_loc = nc.dram_tensor("e_loc", [512], f32)
    e_all = nc.dram_tensor("e_all", [V], f32, addr_space="Shared")

    whh_sb = nc.alloc_sbuf_tensor("whh_sb", [128, H * NJ], f32)     # 64KB/part
    big_sb = nc.alloc_sbuf_tensor("big_sb", [128, 16384], f32)      # gathers->wih->wfc
    xs_T = nc.alloc_sbuf_tensor("xs_T", [128, NK * T], f32)         # 32KB/part
    A_sb = nc.alloc_sbuf_tensor("A_sb", [128, NJ * T], f32)         # 16KB/part
    id_sb = nc.alloc_sbuf_tensor("id_sb", [128, 128], f32)
    ones_p = nc.alloc_sbuf_tensor("ones_p", [128, 1], f32)
    ones_f = nc.alloc_sbuf_tensor("ones_f", [1, 128], f32)
    b_sb = nc.alloc_sbuf_tensor("b_sb", [128, NJ], f32)
    bfc_sb = nc.alloc_sbuf_tensor("bfc_sb", [128, 4], f32)
    idxa_sb = nc.alloc_sbuf_tensor("idxa_sb", [128, 4], i32)
    idxb_sb = nc.alloc_sbuf_tensor("idxb_sb", [128, 4], i32)
    h_all_sb = nc.alloc_sbuf_tensor("h_all_sb", [128, 32], f32)     # 2 parity halves
    h_new_sb = nc.alloc_sbuf_tensor("h_new_sb", [128, 4], f32)
    gates_sb = nc.alloc_sbuf_tensor("gates_sb", [128, 16], f32)
    nl_sb = nc.alloc_sbuf_tensor("nl_sb", [128, 16], f32)
    tmp_sb = nc.alloc_sbuf_tensor("tmp_sb", [128, 8], f32)
    cq_sb = nc.alloc_sbuf_tensor("cq_sb", [128, 4], f32)
    tc_sb = nc.alloc_sbuf_tensor("tc_sb", [128, 4], f32)
    fcl_sb = nc.alloc_sbuf_tensor("fcl_sb", [128, 4], f32)
    exp_sb = nc.alloc_sbuf_tensor("exp_sb", [128, 4], f32)
    esm_sb = nc.alloc_sbuf_tensor("esm_sb", [128, 32], f32)
    osb = nc.alloc_sbuf_tensor("osb", [128, 32], f32)
    red_sb = nc.alloc_sbuf_tensor("red_sb", [128, 1], f32)
    rs_sb = nc.alloc_sbuf_tensor("rs_sb", [1, 1], f32)

    psum = [nc.alloc_psum_tensor(f"ps{j}", [128, 512], f32) for j in range(8)]

    cores = list(range(M))

    # --- static semaphore schedule ---------------------------------------
    PE_TRANS = 64                      # after 64 transposes
    PE_APRE = lambda j: PE_TRANS + j + 1      # after A-precompute col-block j
    PE_STEP = lambda t: 72 + t                # after step-t matvecs (t>=1)
    PE_FC = 584
    PE_SUM = 585
    PE_BC = 586
    G_GATH = 128                               # 8 gathers done
    G_OUT = lambda t: G_GATH + 32 * t + 16     # h_loc out-dma of step t
    G_IN = lambda t: G_GATH + 32 * t + 32      # h_all in-dma of step t
    G_ELOC = G_IN(T - 1) + 4 * 16
    G_ESM = G_ELOC + 32 * 16
    DV_A = 8
    DV_C0 = 9
    DV_H0 = 10
    DV_GATES = lambda t: 3 * t + 8
    DV_C = lambda t: 3 * t + 9
    DV_H = lambda t: 3 * t + 10
    DV_FC = DV_H(T - 1) + 1            # 1544
    DV_RED = DV_FC + 1
    DV_RECIP = DV_FC + 2
    DV_OUT = DV_FC + 3
    AC_COPY = lambda i: i + 1
    AC_A1 = lambda t: 2 * t + 65
    AC_A2 = lambda t: 2 * t + 66
    AC_EXP = AC_A2(T - 1) + 1          # 1089
    LD_WHH, LD_BIAS, LD_BFC, LD_WIH, LD_WFC = 16, 32, 48, 64, 80
    LDI_IDX = 32

    with (
        nc.Block() as block,
        nc.semaphore("ld") as ld,
        nc.semaphore("ldi") as ldi,
        nc.semaphore("gc") as gc,
        nc.semaphore("g16") as g16,
        nc.semaphore("cc") as cc,
        nc.semaphore("pe") as pe,
        nc.semaphore("dv") as dv,
        nc.semaphore("ac") as ac,
        nc.semaphore("vw") as vw,
    ):

        @block.sync
        def _(eng):
            # (p, c) = idx[4p + c]: gather tile c holds timestep t = 4p + c at partition p
            eng.dma_start(out=idxa_sb[:, :], in_=AP(idx_a, 0, [[4, 128], [1, 4]])).then_inc(ldi, 16)
            eng.dma_start(out=idxb_sb[:, :], in_=AP(idx_b, 0, [[4, 128], [1, 4]])).then_inc(ldi, 16)
            eng.dma_start(
                out=AP(whh_sb, 0, [[16384, 128], [1024, 16], [1, 1024]]),
                in_=AP(whh_t, 0, [[1024, 128], [131072, 16], [1, 1024]]),
            ).then_inc(ld, 16)
            eng.dma_start(out=b_sb[:, :], in_=AP(bias_d, 0, [[8, 128], [1, 8]])).then_inc(ld, 16)
            eng.dma_start(out=bfc_sb[:, :], in_=AP(bfc_d, 0, [[4, 128], [1, 4]])).then_inc(ld, 16)
            eng.wait_ge(pe, PE_TRANS)      # transposes done reading big_sb
            eng.dma_start(
                out=AP(big_sb, 0, [[16384, 128], [1024, 16], [1, 1024]]),
                in_=AP(wih_t, 0, [[1024, 128], [131072, 16], [1, 1024]]),
            ).then_inc(ld, 16)
            eng.wait_ge(pe, PE_APRE(7))    # A-precompute done reading big_sb
            eng.dma_start(
                out=AP(big_sb, 0, [[16384, 128], [512, 16], [1, 512]]),
                in_=AP(wfc_t, 0, [[512, 128], [65536, 16], [1, 512]]),
            ).then_inc(ld, 16)

        @block.gpsimd
        def _(eng):
            eng.memset(id_sb[:, :], 1.0).then_inc(gc, 1)
            eng.memset(ones_p[:, :], 1.0).then_inc(gc, 1)
            eng.memset(ones_f[:, :], 1.0).then_inc(gc, 1)
            eng.wait_ge(gc, 3)
            eng.affine_select(
                id_sb[:, :], id_sb[:, :], pattern=[[1, 128]],
                compare_op=mybir.AluOpType.is_equal, fill=0.0,
                base=0, channel_multiplier=-1,
            ).then_inc(gc, 1)
            eng.wait_ge(ldi, LDI_IDX)
            for c in range(4):
                eng.indirect_dma_start(
                    out=big_sb[:, 1024 * c:1024 * c + 1024], out_offset=None,
                    in_=tab_a[:, :],
                    in_offset=bass.IndirectOffsetOnAxis(ap=idxa_sb[:, c:c + 1], axis=0),
                ).then_inc(g16, 16)
            for c in range(4):
                eng.indirect_dma_start(
                    out=big_sb[:, 4096 + 1024 * c:4096 + 1024 * c + 1024], out_offset=None,
                    in_=tab_b[:, :],
                    in_offset=bass.IndirectOffsetOnAxis(ap=idxb_sb[:, c:c + 1], axis=0),
                ).then_inc(g16, 16)
            for t in range(T):
                tq = t % 2
                eng.wait_ge(dv, DV_H0 if t == 0 else DV_H(t))
                eng.dma_start(
                    out=AP(h_loc, 0, [[2, 128], [1, 2]]),
                    in_=h_new_sb[:, 2 * tq:2 * tq + 2],
                ).then_inc(g16, 16)
                eng.wait_ge(g16, G_OUT(t))
                eng.collective_compute(
                    "AllGather", mybir.AluOpType.bypass,
                    replica_groups=[cores],
                    ins=[h_loc[:]], outs=[h_all[:]],
                ).then_inc(cc, 1)
                eng.wait_ge(cc, t + 1)
                eng.dma_start(
                    out=h_all_sb[:, 16 * tq:16 * tq + 16],
                    in_=AP(h_all, 0, [[16, 128], [1, 16]]),
                ).then_inc(g16, 16)
            eng.wait_ge(ac, AC_EXP)
            for j in range(4):
                eng.dma_start(
                    out=AP(e_loc, 128 * j, [[1, 128], [1, 1]]),
                    in_=exp_sb[:, j:j + 1],
                ).then_inc(g16, 16)
            eng.wait_ge(g16, G_ELOC)
            eng.collective_compute(
                "AllGather", mybir.AluOpType.bypass,
                replica_groups=[cores],
                ins=[e_loc[:]], outs=[e_all[:]],
            ).then_inc(cc, 1)
            eng.wait_ge(cc, T + 1)
            for c in range(32):
                eng.dma_start(
                    out=esm_sb[:, c:c + 1],
                    in_=AP(e_all, 128 * c, [[1, 128], [1, 1]]),
                ).then_inc(g16, 16)
            eng.wait_ge(dv, DV_OUT)
            for c in range(32):
                eng.dma_start(
                    out=AP(out_ext, 128 * c, [[1, 128], [1, 1]]),
                    in_=osb[:, c:c + 1],
                ).then_inc(g16, 16)

        @block.tensor
        def _(eng):
            eng.wait_ge(gc, 4)
            eng.wait_ge(g16, G_GATH)
            for i in range(64):                      # i = 16*cp + k
                cp, k = i // 16, i % 16
                if i >= 8:
                    eng.wait_ge(ac, AC_COPY(i - 8))  # bank free after copy
                src_col = (1024 * cp + 128 * k) if k < 8 else (4096 + 1024 * cp + 128 * (k - 8))
                eng.transpose(
                    psum[i % 8][:, 0:128],
                    big_sb[:, src_col:src_col + 128],
                    id_sb[:, :],
                ).then_inc(pe, 1)
            eng.wait_ge(ld, LD_WIH)
            eng.wait_ge(ac, AC_COPY(63))             # xs_T fully written
            for j in range(NJ):
                for c in range(NK):
                    inst = eng.matmul(
                        psum[j][:, 0:512],
                        big_sb[:, 1024 * c + 128 * j:1024 * c + 128 * j + 128],
                        xs_T[:, 512 * c:512 * c + 512],
                        start=(c == 0), stop=(c == NK - 1),
                    )
                    if c == NK - 1:
                        inst.then_inc(pe, 1)
            eng.wait_ge(dv, DV_A)                    # A-adds done: psum 0/1 free
            for t in range(1, T):
                tq, rq = t % 2, (t - 1) % 2
                eng.wait_ge(g16, G_IN(t - 1))
                for j in range(NJ):
                    for k in range(NK):
                        inst = eng.matmul(
                            psum[tq][:, j:j + 1],
                            whh_sb[:, 1024 * k + 128 * j:1024 * k + 128 * j + 128],
                            h_all_sb[:, 16 * rq + k:16 * rq + k + 1],
                            start=(k == 0), stop=(k == NK - 1),
                        )
                        if j == NJ - 1 and k == NK - 1:
                            inst.then_inc(pe, 1)
            eng.wait_ge(g16, G_IN(T - 1))
            eng.wait_ge(ld, LD_WFC)
            fq = (T - 1) % 2
            for j in range(4):
                for k in range(NK):
                    inst = eng.matmul(
                        psum[2][:, j:j + 1],
                        big_sb[:, 512 * k + 128 * j:512 * k + 128 * j + 128],
                        h_all_sb[:, 16 * fq + k:16 * fq + k + 1],
                        start=(k == 0), stop=(k == NK - 1),
                    )
                    if j == 3 and k == NK - 1:
                        inst.then_inc(pe, 1)
            eng.wait_ge(dv, DV_RED)
            eng.matmul(psum[2][0:1, 8:9], ones_p[:, :], red_sb[:, :],
                       start=True, stop=True).then_inc(pe, 1)
            eng.wait_ge(dv, DV_RECIP)
            eng.matmul(psum[2][:, 9:10], ones_f[:, :], rs_sb[:, :],
                       start=True, stop=True).then_inc(pe, 1)

        @block.vector
        def _(eng):
            for j in range(NJ):
                eng.wait_ge(pe, PE_APRE(j))
                eng.tensor_scalar_add(
                    AP(A_sb, j, [[NJ * T, 128], [NJ, T]]),
                    psum[j][:, 0:512],
                    b_sb[:, j:j + 1],
                ).then_inc(dv, 1)
            eng.wait_ge(dv, DV_A)          # A_sb writes retired before self-reads
            # step 0: gates come straight from A (h=0), c_in=0 -> c = sig(i)*tanh(g)
            eng.wait_ge(ac, AC_A1(0))
            eng.tensor_mul(cq_sb[:, 0:2], nl_sb[:, 0:2], nl_sb[:, 4:6]).then_inc(dv, 1)
            eng.wait_ge(ac, AC_A2(0))
            eng.tensor_mul(h_new_sb[:, 0:2], nl_sb[:, 6:8], tc_sb[:, 0:2]).then_inc(dv, 1)
            for t in range(1, T):
                tq = t % 2
                eng.wait_ge(pe, PE_STEP(t))
                eng.tensor_add(
                    gates_sb[:, 8 * tq:8 * tq + 8],
                    psum[tq][:, 0:8],
                    A_sb[:, 8 * t:8 * t + 8],
                ).then_inc(dv, 1)
                eng.wait_ge(ac, AC_A1(t))
                eng.tensor_mul(
                    tmp_sb[:, 4 * tq:4 * tq + 2],
                    nl_sb[:, 8 * tq + 2:8 * tq + 4],
                    AP(xs_T, t, [[NK * T, 128], [512, 2]]),   # c_in = x_t (features 0:256)
                ).then_inc(vw, 1)
                eng.tensor_mul(
                    tmp_sb[:, 4 * tq + 2:4 * tq + 4],
                    nl_sb[:, 8 * tq:8 * tq + 2],
                    nl_sb[:, 8 * tq + 4:8 * tq + 6],
                ).then_inc(vw, 1)
                eng.wait_ge(vw, 2 * t)
                eng.tensor_add(
                    cq_sb[:, 2 * tq:2 * tq + 2],
                    tmp_sb[:, 4 * tq:4 * tq + 2],
                    tmp_sb[:, 4 * tq + 2:4 * tq + 4],
                ).then_inc(dv, 1)
                eng.wait_ge(ac, AC_A2(t))
                eng.tensor_mul(
                    h_new_sb[:, 2 * tq:2 * tq + 2],
                    nl_sb[:, 8 * tq + 6:8 * tq + 8],
                    tc_sb[:, 2 * tq:2 * tq + 2],
                ).then_inc(dv, 1)
            eng.wait_ge(pe, PE_FC)
            eng.tensor_add(fcl_sb[:, :], psum[2][:, 0:4], bfc_sb[:, :]).then_inc(dv, 1)
            eng.wait_ge(g16, G_ESM)
            eng.tensor_reduce(red_sb[:, :], esm_sb[:, :],
                              axis=mybir.AxisListType.X, op=mybir.AluOpType.add).then_inc(dv, 1)
            eng.wait_ge(pe, PE_SUM)
            eng.reciprocal(rs_sb[:, :], psum[2][0:1, 8:9]).then_inc(dv, 1)
            eng.wait_ge(pe, PE_BC)
            eng.tensor_scalar_mul(osb[:, :], esm_sb[:, :], psum[2][:, 9:10]).then_inc(dv, 1)

        @block.scalar
        def _(eng):
            for i in range(64):
                cp, k = i // 16, i % 16
                eng.wait_ge(pe, i + 1)
                # transpose out free index i maps to t = 4i + cp -> stride-4 scatter
                eng.activation(
                    AP(xs_T, 512 * k + cp, [[NK * T, 128], [4, 128]]),
                    psum[i % 8][:, 0:128], Cpy,
                ).then_inc(ac, 1)
            for t in range(T):
                tq = t % 2
                if t == 0:
                    eng.wait_ge(dv, DV_A)
                    g_ap = A_sb
                    base = 0
                else:
                    eng.wait_ge(dv, DV_GATES(t))
                    g_ap = gates_sb
                    base = 8 * tq
                eng.activation(nl_sb[:, 8 * tq:8 * tq + 4], g_ap[:, base:base + 4], Sig)
                eng.activation(nl_sb[:, 8 * tq + 4:8 * tq + 6], g_ap[:, base + 4:base + 6], Tnh)
                eng.activation(nl_sb[:, 8 * tq + 6:8 * tq + 8], g_ap[:, base + 6:base + 8], Sig).then_inc(ac, 1)
                eng.wait_ge(dv, DV_C0 if t == 0 else DV_C(t))
                eng.activation(tc_sb[:, 2 * tq:2 * tq + 2], cq_sb[:, 2 * tq:2 * tq + 2], Tnh).then_inc(ac, 1)
            eng.wait_ge(dv, DV_FC)
            eng.activation(exp_sb[:, :], fcl_sb[:, :], ExpF).then_inc(ac, 1)

    return nc


LAST_EXEC_NS = None


def kernel(**inputs):
    global LAST_EXEC_NS
    from concourse import bass_utils

    if "nc" not in _CACHE:
        _CACHE["nc"] = _build()
    nc = _CACHE["nc"]

    in_maps = _prep_in_maps(inputs)
    trace = bool(int(os.environ.get("KERNEL_TRACE", "0")))
    if trace:
        try:
            res = bass_utils.run_bass_kernel_spmd(nc, in_maps, list(range(M)), trace=True)
        except Exception:
            res = bass_utils.run_bass_kernel_spmd(nc, in_maps, list(range(M)), trace=False)
    else:
        res = bass_utils.run_bass_kernel_spmd(nc, in_maps, list(range(M)), trace=False)
    LAST_EXEC_NS = getattr(res, "exec_time_ns", None)
    out = np.asarray(res.results[0]["out"], dtype=np.float32)
    return out.reshape(1, V)


# revision 26
# speedup vs baseline: 3.2787x; 3.2787x over previous
"""LSTM-like policy net on 8 Trainium2 cores, tensor-parallel over the gate dim.

Per-core shard m owns gate rows [256m, 256m+256) of each gate (i,f,g,o),
h elements [256m, 256m+256), and fc rows [512m, 512m+512).
Per step: local W_hh @ h matvec (128 accumulating matmuls), nonlinearities,
AllGather of the 256-element h shard. x-path (embeddings @ W_ih) is
precomputed for all 512 steps in one parallel pass.
"""

import os
import sys

import ml_dtypes
import numpy as np

if "/opt/trn_rl_repo" not in sys.path:
    sys.path.insert(0, "/opt/trn_rl_repo")

T = 512          # sequence length
D = 2048         # input feature dim (2 x 1024 embeddings)
H = 2048         # hidden dim
L = 1024         # local gate rows per core (4 gates x 256)
V = 4096         # fc output dim
M = 8            # cores
NK = 16          # 128-chunks over D/H
NJ = 8           # 128-chunks over L

_CACHE = {}


def _h_perm():
    # h_all flat index u = 256*q + 2*p + jl  ->  h element 256*q + 128*jl + p
    u = np.arange(H)
    q, r = u // 256, u % 256
    return 256 * q + 128 * (r % 2) + r // 2


def _contract_perm():
    # whh_sb row v = 128*k + kk multiplies rhs element h_all[16*kk + k]
    v = np.arange(H)
    k, kk = v // 128, v % 128
    return _h_perm()[16 * kk + k]


def _prep_in_maps(inputs):
    gz = np.ascontiguousarray(np.asarray(inputs["guesses"]).astype(np.int32).ravel())
    fb = np.ascontiguousarray(np.asarray(inputs["feedbacks"]).astype(np.int32).ravel())
    ge = np.asarray(inputs["guess_embed"], dtype=np.float32)
    fe = np.asarray(inputs["feedback_embed"], dtype=np.float32)
    W_ih = np.asarray(inputs["W_ih"], dtype=np.float32)
    W_hh = np.asarray(inputs["W_hh"], dtype=np.float32)
    bias = (np.asarray(inputs["b_ih"], dtype=np.float32)
            + np.asarray(inputs["b_hh"], dtype=np.float32))
    W_fc = np.asarray(inputs["W_fc"], dtype=np.float32)
    b_fc = np.asarray(inputs["b_fc"], dtype=np.float32)

    cperm = _contract_perm()
    in_maps = []
    for m in range(M):
        # local gate row r = 128*jj + p  ->  global row 2048*(jj//2) + 256*m + 128*(jj%2) + p
        jj = np.arange(NJ)
        rows = (2048 * (jj // 2)[:, None] + 256 * m + 128 * (jj % 2)[:, None]
                + np.arange(128)[None, :]).ravel()
        Wih_sh = W_ih[rows]            # [1024, 2048]
        Whh_sh = W_hh[rows]            # [1024, 2048]
        b_sh = np.ascontiguousarray(bias[rows])

        # x-space feature permutation: own embedding half rolled so this
        # core's 256 c_in features land at positions 0:256
        own = ge if m < 4 else fe
        oth = fe if m < 4 else ge
        own_base = 0 if m < 4 else 1024
        roll = (np.arange(1024) + 256 * (m % 4)) % 1024
        perm = np.concatenate([own_base + roll, (1024 - own_base) + np.arange(1024)])

        in_maps.append({
            "idx_a": gz if m < 4 else fb,
            "idx_b": fb if m < 4 else gz,
            "tab_a": np.ascontiguousarray(own[:, roll]),
            "tab_b": np.ascontiguousarray(oth),
            "wih_t": np.ascontiguousarray(Wih_sh[:, perm].T),      # [2048, 1024]
            "whh_t": np.ascontiguousarray(Whh_sh[:, cperm].T).astype(ml_dtypes.bfloat16),
            # reordered so a contiguous [[8,128],[1,8]] load puts b_sh[128j+p] at (p,j)
            "bias": np.ascontiguousarray(b_sh.reshape(8, 128).T.ravel()),
            "wfc_t": np.ascontiguousarray(W_fc[512 * m:512 * m + 512][:, cperm].T).astype(ml_dtypes.bfloat16),
            "bfc": np.ascontiguousarray(
                b_fc[512 * m:512 * m + 512].reshape(4, 128).T.ravel()),
        })
    return in_maps


def _build():
    from concourse import bass, mybir

    f32 = mybir.dt.float32
    bf16 = mybir.dt.bfloat16
    i32 = mybir.dt.int32
    Sig = mybir.ActivationFunctionType.Sigmoid
    Tnh = mybir.ActivationFunctionType.Tanh
    ExpF = mybir.ActivationFunctionType.Exp
    Cpy = mybir.ActivationFunctionType.Copy
    AP = bass.AP

    nc = bass.Bass(target_bir_lowering=False, debug=False)

    idx_a = nc.declare_dram_parameter("idx_a", [T], i32, isOutput=False)
    idx_b = nc.declare_dram_parameter("idx_b", [T], i32, isOutput=False)
    tab_a = nc.declare_dram_parameter("tab_a", [4097, 1024], f32, isOutput=False)
    tab_b = nc.declare_dram_parameter("tab_b", [4097, 1024], f32, isOutput=False)
    wih_t = nc.declare_dram_parameter("wih_t", [D, L], f32, isOutput=False)
    whh_t = nc.declare_dram_parameter("whh_t", [H, L], bf16, isOutput=False)
    bias_d = nc.declare_dram_parameter("bias", [L], f32, isOutput=False)
    wfc_t = nc.declare_dram_parameter("wfc_t", [H, 512], bf16, isOutput=False)
    bfc_d = nc.declare_dram_parameter("bfc", [512], f32, isOutput=False)
    out_ext = nc.declare_dram_parameter("out", [V], f32, isOutput=True)

    h_loc = nc.dram_tensor("h_loc", [256], bf16)
    h_all = nc.dram_tensor("h_all", [H], bf16, addr_space="Shared")
    e_loc = nc.dram_tensor("e_loc", [512], f32)
    e_all = nc.dram_tensor("e_all", [V], f32, addr_space="Shared")

    whh_sb = nc.alloc_sbuf_tensor("whh_sb", [128, H * NJ], bf16)    # 32KB/part
    wfc_sb = nc.alloc_sbuf_tensor("wfc_sb", [128, 8192], bf16)      # 16KB/part
    big_sb = nc.alloc_sbuf_tensor("big_sb", [128, 16384], f32)      # gathers->wih->wfc
    xs_T = nc.alloc_sbuf_tensor("xs_T", [128, NK * T], f32)         # 32KB/part
    A_sb = nc.alloc_sbuf_tensor("A_sb", [128, NJ * T], f32)         # 16KB/part
    id_sb = nc.alloc_sbuf_tensor("id_sb", [128, 128], f32)
    ones_p = nc.alloc_sbuf_tensor("ones_p", [128, 1], f32)
    ones_f = nc.alloc_sbuf_tensor("ones_f", [1, 128], f32)
    b_sb = nc.alloc_sbuf_tensor("b_sb", [128, NJ], f32)
    bfc_sb = nc.alloc_sbuf_tensor("bfc_sb", [128, 4], f32)
    idxa_sb = nc.alloc_sbuf_tensor("idxa_sb", [128, 4], i32)
    idxb_sb = nc.alloc_sbuf_tensor("idxb_sb", [128, 4], i32)
    h_all_sb = nc.alloc_sbuf_tensor("h_all_sb", [128, 32], bf16)    # 2 parity halves
    h_new_sb = nc.alloc_sbuf_tensor("h_new_sb", [128, 4], bf16)
    gates_sb = nc.alloc_sbuf_tensor("gates_sb", [128, 16], f32)
    nl_sb = nc.alloc_sbuf_tensor("nl_sb", [128, 16], f32)
    tmp_sb = nc.alloc_sbuf_tensor("tmp_sb", [128, 8], f32)
    cq_sb = nc.alloc_sbuf_tensor("cq_sb", [128, 4], f32)
    tc_sb = nc.alloc_sbuf_tensor("tc_sb", [128, 4], f32)
    fcl_sb = nc.alloc_sbuf_tensor("fcl_sb", [128, 4], f32)
    exp_sb = nc.alloc_sbuf_tensor("exp_sb", [128, 4], f32)
    esm_sb = nc.alloc_sbuf_tensor("esm_sb", [128, 32], f32)
    osb = nc.alloc_sbuf_tensor("osb", [128, 32], f32)
    red_sb = nc.alloc_sbuf_tensor("red_sb", [128, 1], f32)
    rs_sb = nc.alloc_sbuf_tensor("rs_sb", [1, 1], f32)

    psum = [nc.alloc_psum_tensor(f"ps{j}", [128, 512], f32) for j in range(8)]

    cores = list(range(M))

    # --- static semaphore schedule ---------------------------------------
    PE_TRANS = 64                      # after 64 transposes
    PE_APRE = lambda j: PE_TRANS + j + 1      # after A-precompute col-block j
    PE_STEP = lambda t: 72 + t                # after step-t matvecs (t>=1)
    PE_FC = 584
    PE_SUM = 585
    PE_BC = 586
    G_GATH = 128                               # 8 gathers done
    G_OUT = lambda t: G_GATH + 32 * t + 16     # h_loc out-dma of step t
    G_IN = lambda t: G_GATH + 32 * t + 32      # h_all in-dma of step t
    G_ELOC = G_IN(T - 1) + 4 * 16
    G_ESM = G_ELOC + 32 * 16
    DV_A = 8
    DV_C0 = 9
    DV_H0 = 10
    DV_GATES = lambda t: 3 * t + 8
    DV_C = lambda t: 3 * t + 9
    DV_H = lambda t: 3 * t + 10
    DV_FC = DV_H(T - 1) + 1            # 1544
    DV_RED = DV_FC + 1
    DV_RECIP = DV_FC + 2
    DV_OUT = DV_FC + 3
    AC_COPY = lambda i: i + 1
    AC_A1 = lambda t: 2 * t + 65
    AC_A2 = lambda t: 2 * t + 66
    AC_EXP = AC_A2(T - 1) + 1          # 1089
    LD_WHH, LD_BIAS, LD_BFC, LD_WFC, LD_WIH = 16, 32, 48, 64, 80
    LDI_IDX = 32

    with (
        nc.Block() as block,
        nc.semaphore("ld") as ld,
        nc.semaphore("ldi") as ldi,
        nc.semaphore("gc") as gc,
        nc.semaphore("g16") as g16,
        nc.semaphore("cc") as cc,
        nc.semaphore("pe") as pe,
        nc.semaphore("dv") as dv,
        nc.semaphore("ac") as ac,
        nc.semaphore("vw") as vw,
    ):

        @block.sync
        def _(eng):
            # (p, c) = idx[4p + c]: gather tile c holds timestep t = 4p + c at partition p
            eng.dma_start(out=idxa_sb[:, :], in_=AP(idx_a, 0, [[4, 128], [1, 4]])).then_inc(ldi, 16)
            eng.dma_start(out=idxb_sb[:, :], in_=AP(idx_b, 0, [[4, 128], [1, 4]])).then_inc(ldi, 16)
            eng.dma_start(
                out=AP(whh_sb, 0, [[16384, 128], [1024, 16], [1, 1024]]),
                in_=AP(whh_t, 0, [[1024, 128], [131072, 16], [1, 1024]]),
            ).then_inc(ld, 16)
            eng.dma_start(out=b_sb[:, :], in_=AP(bias_d, 0, [[8, 128], [1, 8]])).then_inc(ld, 16)
            eng.dma_start(out=bfc_sb[:, :], in_=AP(bfc_d, 0, [[4, 128], [1, 4]])).then_inc(ld, 16)
            eng.dma_start(
                out=AP(wfc_sb, 0, [[8192, 128], [512, 16], [1, 512]]),
                in_=AP(wfc_t, 0, [[512, 128], [65536, 16], [1, 512]]),
            ).then_inc(ld, 16)
            eng.wait_ge(pe, PE_TRANS)      # transposes done reading big_sb
            eng.dma_start(
                out=AP(big_sb, 0, [[16384, 128], [1024, 16], [1, 1024]]),
                in_=AP(wih_t, 0, [[1024, 128], [131072, 16], [1, 1024]]),
            ).then_inc(ld, 16)

        @block.gpsimd
        def _(eng):
            eng.memset(id_sb[:, :], 1.0).then_inc(gc, 1)
            eng.memset(ones_p[:, :], 1.0).then_inc(gc, 1)
            eng.memset(ones_f[:, :], 1.0).then_inc(gc, 1)
            eng.wait_ge(gc, 3)
            eng.affine_select(
                id_sb[:, :], id_sb[:, :], pattern=[[1, 128]],
                compare_op=mybir.AluOpType.is_equal, fill=0.0,
                base=0, channel_multiplier=-1,
            ).then_inc(gc, 1)
            eng.wait_ge(ldi, LDI_IDX)
            for c in range(4):
                eng.indirect_dma_start(
                    out=big_sb[:, 1024 * c:1024 * c + 1024], out_offset=None,
                    in_=tab_a[:, :],
                    in_offset=bass.IndirectOffsetOnAxis(ap=idxa_sb[:, c:c + 1], axis=0),
                ).then_inc(g16, 16)
            for c in range(4):
                eng.indirect_dma_start(
                    out=big_sb[:, 4096 + 1024 * c:4096 + 1024 * c + 1024], out_offset=None,
                    in_=tab_b[:, :],
                    in_offset=bass.IndirectOffsetOnAxis(ap=idxb_sb[:, c:c + 1], axis=0),
                ).then_inc(g16, 16)
            for t in range(T):
                tq = t % 2
                eng.wait_ge(dv, DV_H0 if t == 0 else DV_H(t))
                eng.dma_start(
                    out=AP(h_loc, 0, [[2, 128], [1, 2]]),
                    in_=h_new_sb[:, 2 * tq:2 * tq + 2],
                ).then_inc(g16, 16)
                eng.wait_ge(g16, G_OUT(t))
                eng.collective_compute(
                    "AllGather", mybir.AluOpType.bypass,
                    replica_groups=[cores],
                    ins=[h_loc[:]], outs=[h_all[:]],
                ).then_inc(cc, 1)
                eng.wait_ge(cc, t + 1)
                eng.dma_start(
                    out=h_all_sb[:, 16 * tq:16 * tq + 16],
                    in_=AP(h_all, 0, [[16, 128], [1, 16]]),
                ).then_inc(g16, 16)
            eng.wait_ge(ac, AC_EXP)
            for j in range(4):
                eng.dma_start(
                    out=AP(e_loc, 128 * j, [[1, 128], [1, 1]]),
                    in_=exp_sb[:, j:j + 1],
                ).then_inc(g16, 16)
            eng.wait_ge(g16, G_ELOC)
            eng.collective_compute(
                "AllGather", mybir.AluOpType.bypass,
                replica_groups=[cores],
                ins=[e_loc[:]], outs=[e_all[:]],
            ).then_inc(cc, 1)
            eng.wait_ge(cc, T + 1)
            for c in range(32):
                eng.dma_start(
                    out=esm_sb[:, c:c + 1],
                    in_=AP(e_all, 128 * c, [[1, 128], [1, 1]]),
                ).then_inc(g16, 16)
            eng.wait_ge(dv, DV_OUT)
            for c in range(32):
                eng.dma_start(
                    out=AP(out_ext, 128 * c, [[1, 128], [1, 1]]),
                    in_=osb[:, c:c + 1],
                ).then_inc(g16, 16)

        @block.tensor
        def _(eng):
            eng.wait_ge(gc, 4)
            eng.wait_ge(g16, G_GATH)
            for i in range(64):                      # i = 16*cp + k
                cp, k = i // 16, i % 16
                if i >= 8:
                    eng.wait_ge(ac, AC_COPY(i - 8))  # bank free after copy
                src_col = (1024 * cp + 128 * k) if k < 8 else (4096 + 1024 * cp + 128 * (k - 8))
                eng.transpose(
                    psum[i % 8][:, 0:128],
                    big_sb[:, src_col:src_col + 128],
                    id_sb[:, :],
                ).then_inc(pe, 1)
            eng.wait_ge(ld, LD_WIH)
            eng.wait_ge(ac, AC_COPY(63))             # xs_T fully written
            for j in range(NJ):
                for c in range(NK):
                    inst = eng.matmul(
                        psum[j][:, 0:512],
                        big_sb[:, 1024 * c + 128 * j:1024 * c + 128 * j + 128],
                        xs_T[:, 512 * c:512 * c + 512],
                        start=(c == 0), stop=(c == NK - 1),
                    )
                    if c == NK - 1:
                        inst.then_inc(pe, 1)
            eng.wait_ge(dv, DV_A)                    # A-adds done: psum 0/1 free
            for t in range(1, T):
                tq, rq = t % 2, (t - 1) % 2
                eng.wait_ge(g16, G_IN(t - 1))
                for j in range(NJ):
                    for k in range(NK):
                        inst = eng.matmul(
                            psum[tq][:, j:j + 1],
                            whh_sb[:, 1024 * k + 128 * j:1024 * k + 128 * j + 128],
                            h_all_sb[:, 16 * rq + k:16 * rq + k + 1],
                            start=(k == 0), stop=(k == NK - 1),
                        )
                        if j == NJ - 1 and k == NK - 1:
                            inst.then_inc(pe, 1)
            eng.wait_ge(g16, G_IN(T - 1))
            eng.wait_ge(ld, LD_WFC)
            fq = (T - 1) % 2
            for j in range(4):
                for k in range(NK):
                    inst = eng.matmul(
                        psum[2][:, j:j + 1],
                        wfc_sb[:, 512 * k + 128 * j:512 * k + 128 * j + 128],
                        h_all_sb[:, 16 * fq + k:16 * fq + k + 1],
                        start=(k == 0), stop=(k == NK - 1),
                    )
                    if j == 3 and k == NK - 1:
                        inst.then_inc(pe, 1)
            eng.wait_ge(dv, DV_RED)
            eng.matmul(psum[2][0:1, 8:9], ones_p[:, :], red_sb[:, :],
                       start=True, stop=True).then_inc(pe, 1)
            eng.wait_ge(dv, DV_RECIP)
            eng.matmul(psum[2][:, 9:10], ones_f[:, :], rs_sb[:, :],
                       start=True, stop=True).then_inc(pe, 1)

        @block.vector
        def _(eng):
            for j in range(NJ):
                eng.wait_ge(pe, PE_APRE(j))
                eng.tensor_scalar_add(
                    AP(A_sb, j, [[NJ * T, 128], [NJ, T]]),
                    psum[j][:, 0:512],
                    b_sb[:, j:j + 1],
                ).then_inc(dv, 1)
            eng.wait_ge(dv, DV_A)          # A_sb writes retired before self-reads
            # step 0: gates come straight from A (h=0), c_in=0 -> c = sig(i)*tanh(g)
            eng.wait_ge(ac, AC_A1(0))
            eng.tensor_mul(cq_sb[:, 0:2], nl_sb[:, 0:2], nl_sb[:, 4:6]).then_inc(dv, 1)
            eng.wait_ge(ac, AC_A2(0))
            eng.tensor_mul(h_new_sb[:, 0:2], nl_sb[:, 6:8], tc_sb[:, 0:2]).then_inc(dv, 1)
            for t in range(1, T):
                tq = t % 2
                eng.wait_ge(pe, PE_STEP(t))
                eng.tensor_add(
                    gates_sb[:, 8 * tq:8 * tq + 8],
                    psum[tq][:, 0:8],
                    A_sb[:, 8 * t:8 * t + 8],
                ).then_inc(dv, 1)
                eng.wait_ge(ac, AC_A1(t))
                eng.tensor_mul(
                    tmp_sb[:, 4 * tq:4 * tq + 2],
                    nl_sb[:, 8 * tq + 2:8 * tq + 4],
                    AP(xs_T, t, [[NK * T, 128], [512, 2]]),   # c_in = x_t (features 0:256)
                ).then_inc(vw, 1)
                eng.tensor_mul(
                    tmp_sb[:, 4 * tq + 2:4 * tq + 4],
                    nl_sb[:, 8 * tq:8 * tq + 2],
                    nl_sb[:, 8 * tq + 4:8 * tq + 6],
                ).then_inc(vw, 1)
                eng.wait_ge(vw, 2 * t)
                eng.tensor_add(
                    cq_sb[:, 2 * tq:2 * tq + 2],
                    tmp_sb[:, 4 * tq:4 * tq + 2],
                    tmp_sb[:, 4 * tq + 2:4 * tq + 4],
                ).then_inc(dv, 1)
                eng.wait_ge(ac, AC_A2(t))
                eng.tensor_mul(
                    h_new_sb[:, 2 * tq:2 * tq + 2],
                    nl_sb[:, 8 * tq + 6:8 * tq + 8],
                    tc_sb[:, 2 * tq:2 * tq + 2],
                ).then_inc(dv, 1)
            eng.wait_ge(pe, PE_FC)
            eng.tensor_add(fcl_sb[:, :], psum[2][:, 0:4], bfc_sb[:, :]).then_inc(dv, 1)
            eng.wait_ge(g16, G_ESM)
            eng.tensor_reduce(red_sb[:, :], esm_sb[:, :],
                              axis=mybir.AxisListType.X, op=mybir.AluOpType.add).then_inc(dv, 1)
            eng.wait_ge(pe, PE_SUM)
            eng.reciprocal(rs_sb[:, :], psum[2][0:1, 8:9]).then_inc(dv, 1)
            eng.wait_ge(pe, PE_BC)
            eng.tensor_scalar_mul(osb[:, :], esm_sb[:, :], psum[2][:, 9:10]).then_inc(dv, 1)

        @block.scalar
        def _(eng):
            for i in range(64):
                cp, k = i // 16, i % 16
                eng.wait_ge(pe, i + 1)
                # transpose out free index i maps to t = 4i + cp -> stride-4 scatter
                eng.activation(
                    AP(xs_T, 512 * k + cp, [[NK * T, 128], [4, 128]]),
                    psum[i % 8][:, 0:128], Cpy,
                ).then_inc(ac, 1)
            for t in range(T):
                tq = t % 2
                if t == 0:
                    eng.wait_ge(dv, DV_A)
                    g_ap = A_sb
                    base = 0
                else:
                    eng.wait_ge(dv, DV_GATES(t))
                    g_ap = gates_sb
                    base = 8 * tq
                eng.activation(nl_sb[:, 8 * tq:8 * tq + 4], g_ap[:, base:base + 4], Sig)
                eng.activation(nl_sb[:, 8 * tq + 4:8 * tq + 6], g_ap[:, base + 4:base + 6], Tnh)
                eng.activation(nl_sb[:, 8 * tq + 6:8 * tq + 8], g_ap[:, base + 6:base + 8], Sig).then_inc(ac, 1)
                eng.wait_ge(dv, DV_C0 if t == 0 else DV_C(t))
                eng.activation(tc_sb[:, 2 * tq:2 * tq + 2], cq_sb[:, 2 * tq:2 * tq + 2], Tnh).then_inc(ac, 1)
            eng.wait_ge(dv, DV_FC)
            eng.activation(exp_sb[:, :], fcl_sb[:, :], ExpF).then_inc(ac, 1)

    return nc


LAST_EXEC_NS = None


def kernel(**inputs):
    global LAST_EXEC_NS
    from concourse import bass_utils

    if "nc" not in _CACHE:
        _CACHE["nc"] = _build()
    nc = _CACHE["nc"]

    in_maps = _prep_in_maps(inputs)
    trace = bool(int(os.environ.get("KERNEL_TRACE", "0")))
    if trace:
        try:
            res = bass_utils.run_bass_kernel_spmd(nc, in_maps, list(range(M)), trace=True)
        except Exception:
            res = bass_utils.run_bass_kernel_spmd(nc, in_maps, list(range(M)), trace=False)
    else:
        res = bass_utils.run_bass_kernel_spmd(nc, in_maps, list(range(M)), trace=False)
    LAST_EXEC_NS = getattr(res, "exec_time_ns", None)
    out = np.asarray(res.results[0]["out"], dtype=np.float32)
    return out.reshape(1, V)


# revision 28
# speedup vs baseline: 3.3179x; 1.0120x over previous
"""LSTM-like policy net on 8 Trainium2 cores, tensor-parallel over the gate dim.

Per-core shard m owns gate rows [256m, 256m+256) of each gate (i,f,g,o),
h elements [256m, 256m+256), and fc rows [512m, 512m+512).
Per step: local W_hh @ h matvec (128 accumulating matmuls), nonlinearities,
AllGather of the 256-element h shard. x-path (embeddings @ W_ih) is
precomputed for all 512 steps in one parallel pass.
"""

import os
import sys

import ml_dtypes
import numpy as np

if "/opt/trn_rl_repo" not in sys.path:
    sys.path.insert(0, "/opt/trn_rl_repo")

T = 512          # sequence length
D = 2048         # input feature dim (2 x 1024 embeddings)
H = 2048         # hidden dim
L = 1024         # local gate rows per core (4 gates x 256)
V = 4096         # fc output dim
M = 8            # cores
NK = 16          # 128-chunks over D/H
NJ = 8           # 128-chunks over L

_CACHE = {}


def _h_perm():
    # h_all flat index u = 256*q + 2*p + jl  ->  h element 256*q + 128*jl + p
    u = np.arange(H)
    q, r = u // 256, u % 256
    return 256 * q + 128 * (r % 2) + r // 2


def _contract_perm():
    # whh_sb row v = 128*k + kk multiplies rhs element h_all[16*kk + k]
    v = np.arange(H)
    k, kk = v // 128, v % 128
    return _h_perm()[16 * kk + k]


def _prep_in_maps(inputs):
    gz = np.ascontiguousarray(np.asarray(inputs["guesses"]).astype(np.int32).ravel())
    fb = np.ascontiguousarray(np.asarray(inputs["feedbacks"]).astype(np.int32).ravel())
    ge = np.asarray(inputs["guess_embed"], dtype=np.float32)
    fe = np.asarray(inputs["feedback_embed"], dtype=np.float32)
    W_ih = np.asarray(inputs["W_ih"], dtype=np.float32)
    W_hh = np.asarray(inputs["W_hh"], dtype=np.float32)
    bias = (np.asarray(inputs["b_ih"], dtype=np.float32)
            + np.asarray(inputs["b_hh"], dtype=np.float32))
    W_fc = np.asarray(inputs["W_fc"], dtype=np.float32)
    b_fc = np.asarray(inputs["b_fc"], dtype=np.float32)

    cperm = _contract_perm()
    in_maps = []
    for m in range(M):
        # local gate row r = 128*jj + p  ->  global row 2048*(jj//2) + 256*m + 128*(jj%2) + p
        jj = np.arange(NJ)
        rows = (2048 * (jj // 2)[:, None] + 256 * m + 128 * (jj % 2)[:, None]
                + np.arange(128)[None, :]).ravel()
        Wih_sh = W_ih[rows]            # [1024, 2048]
        Whh_sh = W_hh[rows]            # [1024, 2048]
        b_sh = np.ascontiguousarray(bias[rows])

        # x-space feature permutation: own embedding half rolled so this
        # core's 256 c_in features land at positions 0:256
        own = ge if m < 4 else fe
        oth = fe if m < 4 else ge
        own_base = 0 if m < 4 else 1024
        roll = (np.arange(1024) + 256 * (m % 4)) % 1024
        perm = np.concatenate([own_base + roll, (1024 - own_base) + np.arange(1024)])

        in_maps.append({
            "idx_a": gz if m < 4 else fb,
            "idx_b": fb if m < 4 else gz,
            "tab_a": np.ascontiguousarray(own[:, roll]),
            "tab_b": np.ascontiguousarray(oth),
            "wih_t": np.ascontiguousarray(Wih_sh[:, perm].T),      # [2048, 1024]
            "whh_t": np.ascontiguousarray(Whh_sh[:, cperm].T).astype(ml_dtypes.bfloat16),
            # reordered so a contiguous [[8,128],[1,8]] load puts b_sh[128j+p] at (p,j)
            "bias": np.ascontiguousarray(b_sh.reshape(8, 128).T.ravel()),
            "wfc_t": np.ascontiguousarray(W_fc[512 * m:512 * m + 512][:, cperm].T).astype(ml_dtypes.bfloat16),
            "bfc": np.ascontiguousarray(
                b_fc[512 * m:512 * m + 512].reshape(4, 128).T.ravel()),
        })
    return in_maps


def _build():
    from concourse import bass, mybir

    f32 = mybir.dt.float32
    bf16 = mybir.dt.bfloat16
    i32 = mybir.dt.int32
    Sig = mybir.ActivationFunctionType.Sigmoid
    Tnh = mybir.ActivationFunctionType.Tanh
    ExpF = mybir.ActivationFunctionType.Exp
    Cpy = mybir.ActivationFunctionType.Copy
    AP = bass.AP

    nc = bass.Bass(target_bir_lowering=False, debug=False)

    idx_a = nc.declare_dram_parameter("idx_a", [T], i32, isOutput=False)
    idx_b = nc.declare_dram_parameter("idx_b", [T], i32, isOutput=False)
    tab_a = nc.declare_dram_parameter("tab_a", [4097, 1024], f32, isOutput=False)
    tab_b = nc.declare_dram_parameter("tab_b", [4097, 1024], f32, isOutput=False)
    wih_t = nc.declare_dram_parameter("wih_t", [D, L], f32, isOutput=False)
    whh_t = nc.declare_dram_parameter("whh_t", [H, L], bf16, isOutput=False)
    bias_d = nc.declare_dram_parameter("bias", [L], f32, isOutput=False)
    wfc_t = nc.declare_dram_parameter("wfc_t", [H, 512], bf16, isOutput=False)
    bfc_d = nc.declare_dram_parameter("bfc", [512], f32, isOutput=False)
    out_ext = nc.declare_dram_parameter("out", [V], f32, isOutput=True)

    h_loc = nc.dram_tensor("h_loc", [256], bf16)
    h_all = nc.dram_tensor("h_all", [H], bf16, addr_space="Shared")
    e_loc = nc.dram_tensor("e_loc", [512], f32)
    e_all = nc.dram_tensor("e_all", [V], f32, addr_space="Shared")

    whh_sb = nc.alloc_sbuf_tensor("whh_sb", [128, H * NJ], bf16)    # 32KB/part
    wfc_sb = nc.alloc_sbuf_tensor("wfc_sb", [128, 8192], bf16)      # 16KB/part
    big_sb = nc.alloc_sbuf_tensor("big_sb", [128, 16384], f32)      # gathers->wih->wfc
    xs_T = nc.alloc_sbuf_tensor("xs_T", [128, NK * T], f32)         # 32KB/part
    A_sb = nc.alloc_sbuf_tensor("A_sb", [128, NJ * T], f32)         # 16KB/part
    id_sb = nc.alloc_sbuf_tensor("id_sb", [128, 128], f32)
    ones_p = nc.alloc_sbuf_tensor("ones_p", [128, 1], f32)
    ones_f = nc.alloc_sbuf_tensor("ones_f", [1, 128], f32)
    b_sb = nc.alloc_sbuf_tensor("b_sb", [128, NJ], f32)
    bfc_sb = nc.alloc_sbuf_tensor("bfc_sb", [128, 4], f32)
    idxa_sb = nc.alloc_sbuf_tensor("idxa_sb", [128, 4], i32)
    idxb_sb = nc.alloc_sbuf_tensor("idxb_sb", [128, 4], i32)
    h_all_sb = nc.alloc_sbuf_tensor("h_all_sb", [128, 32], bf16)    # 2 parity halves
    h_new_sb = nc.alloc_sbuf_tensor("h_new_sb", [128, 4], bf16)
    gates_sb = nc.alloc_sbuf_tensor("gates_sb", [128, 16], f32)
    nl_sb = nc.alloc_sbuf_tensor("nl_sb", [128, 16], f32)
    tmp_sb = nc.alloc_sbuf_tensor("tmp_sb", [128, 8], f32)
    cq_sb = nc.alloc_sbuf_tensor("cq_sb", [128, 4], f32)
    tc_sb = nc.alloc_sbuf_tensor("tc_sb", [128, 4], f32)
    fcl_sb = nc.alloc_sbuf_tensor("fcl_sb", [128, 4], f32)
    exp_sb = nc.alloc_sbuf_tensor("exp_sb", [128, 4], f32)
    esm_sb = nc.alloc_sbuf_tensor("esm_sb", [128, 32], f32)
    osb = nc.alloc_sbuf_tensor("osb", [128, 32], f32)
    red_sb = nc.alloc_sbuf_tensor("red_sb", [128, 1], f32)
    rs_sb = nc.alloc_sbuf_tensor("rs_sb", [1, 1], f32)

    psum = [nc.alloc_psum_tensor(f"ps{j}", [128, 512], f32) for j in range(8)]

    cores = list(range(M))

    # --- static semaphore schedule ---------------------------------------
    PE_TRANS = 64                      # after 64 transposes
    PE_APRE = lambda j: PE_TRANS + j + 1      # after A-precompute col-block j
    PE_STEP = lambda t: 72 + t                # after step-t matvecs (t>=1)
    PE_FC = 584
    PE_SUM = 585
    PE_BC = 586
    G_GATH = 128                               # 8 gathers done
    G_OUT = lambda t: G_GATH + 32 * t + 16     # h_loc out-dma of step t
    G_IN = lambda t: G_GATH + 32 * t + 32      # h_all in-dma of step t
    G_ELOC = G_IN(T - 1) + 4 * 16
    G_ESM = G_ELOC + 32 * 16
    DV_A = 8
    DV_C0 = 9
    DV_H0 = 10
    DV_GATES = lambda t: 3 * t + 8
    DV_C = lambda t: 3 * t + 9
    DV_H = lambda t: 3 * t + 10
    DV_FC = DV_H(T - 1) + 1            # 1544
    DV_RED = DV_FC + 1
    DV_RECIP = DV_FC + 2
    DV_OUT = DV_FC + 3
    AC_COPY = lambda i: i + 1
    AC_A1 = lambda t: 2 * t + 65
    AC_A2 = lambda t: 2 * t + 66
    AC_EXP = AC_A2(T - 1) + 1          # 1089
    LD_WHH, LD_BIAS, LD_BFC, LD_WFC, LD_WIH = 16, 32, 48, 64, 80
    LDI_IDX = 32

    with (
        nc.Block() as block,
        nc.semaphore("ld") as ld,
        nc.semaphore("ldi") as ldi,
        nc.semaphore("gc") as gc,
        nc.semaphore("g16") as g16,
        nc.semaphore("cc") as cc,
        nc.semaphore("pe") as pe,
        nc.semaphore("dv") as dv,
        nc.semaphore("ac") as ac,
        nc.semaphore("vw") as vw,
    ):

        @block.sync
        def _(eng):
            # (p, c) = idx[4p + c]: gather tile c holds timestep t = 4p + c at partition p
            eng.dma_start(out=idxa_sb[:, :], in_=AP(idx_a, 0, [[4, 128], [1, 4]])).then_inc(ldi, 16)
            eng.dma_start(out=idxb_sb[:, :], in_=AP(idx_b, 0, [[4, 128], [1, 4]])).then_inc(ldi, 16)
            eng.dma_start(
                out=AP(whh_sb, 0, [[16384, 128], [1024, 16], [1, 1024]]),
                in_=AP(whh_t, 0, [[1024, 128], [131072, 16], [1, 1024]]),
            ).then_inc(ld, 16)
            eng.dma_start(out=b_sb[:, :], in_=AP(bias_d, 0, [[8, 128], [1, 8]])).then_inc(ld, 16)
            eng.dma_start(out=bfc_sb[:, :], in_=AP(bfc_d, 0, [[4, 128], [1, 4]])).then_inc(ld, 16)
            eng.dma_start(
                out=AP(wfc_sb, 0, [[8192, 128], [512, 16], [1, 512]]),
                in_=AP(wfc_t, 0, [[512, 128], [65536, 16], [1, 512]]),
            ).then_inc(ld, 16)
            eng.wait_ge(pe, PE_TRANS)      # transposes done reading big_sb
            eng.dma_start(
                out=AP(big_sb, 0, [[16384, 128], [1024, 16], [1, 1024]]),
                in_=AP(wih_t, 0, [[1024, 128], [131072, 16], [1, 1024]]),
            ).then_inc(ld, 16)

        @block.gpsimd
        def _(eng):
            eng.memset(id_sb[:, :], 1.0).then_inc(gc, 1)
            eng.memset(ones_p[:, :], 1.0).then_inc(gc, 1)
            eng.memset(ones_f[:, :], 1.0).then_inc(gc, 1)
            eng.wait_ge(gc, 3)
            eng.affine_select(
                id_sb[:, :], id_sb[:, :], pattern=[[1, 128]],
                compare_op=mybir.AluOpType.is_equal, fill=0.0,
                base=0, channel_multiplier=-1,
            ).then_inc(gc, 1)
            eng.wait_ge(ldi, LDI_IDX)
            for c in range(4):
                eng.indirect_dma_start(
                    out=big_sb[:, 1024 * c:1024 * c + 1024], out_offset=None,
                    in_=tab_a[:, :],
                    in_offset=bass.IndirectOffsetOnAxis(ap=idxa_sb[:, c:c + 1], axis=0),
                ).then_inc(g16, 16)
            for c in range(4):
                eng.indirect_dma_start(
                    out=big_sb[:, 4096 + 1024 * c:4096 + 1024 * c + 1024], out_offset=None,
                    in_=tab_b[:, :],
                    in_offset=bass.IndirectOffsetOnAxis(ap=idxb_sb[:, c:c + 1], axis=0),
                ).then_inc(g16, 16)
            for t in range(T):
                tq = t % 2
                eng.wait_ge(dv, DV_H0 if t == 0 else DV_H(t))
                eng.dma_start(
                    out=AP(h_loc, 0, [[2, 128], [1, 2]]),
                    in_=h_new_sb[:, 2 * tq:2 * tq + 2],
                ).then_inc(g16, 16)
                eng.wait_ge(g16, G_OUT(t))
                eng.collective_compute(
                    "AllGather", mybir.AluOpType.bypass,
                    replica_groups=[cores],
                    ins=[h_loc[:]], outs=[h_all[:]],
                ).then_inc(cc, 1)
                eng.wait_ge(cc, t + 1)
                eng.dma_start(
                    out=h_all_sb[:, 16 * tq:16 * tq + 16],
                    in_=AP(h_all, 0, [[16, 128], [1, 16]]),
                ).then_inc(g16, 16)
            eng.wait_ge(ac, AC_EXP)
            for j in range(4):
                eng.dma_start(
                    out=AP(e_loc, 128 * j, [[1, 128], [1, 1]]),
                    in_=exp_sb[:, j:j + 1],
                ).then_inc(g16, 16)
            eng.wait_ge(g16, G_ELOC)
            eng.collective_compute(
                "AllGather", mybir.AluOpType.bypass,
                replica_groups=[cores],
                ins=[e_loc[:]], outs=[e_all[:]],
            ).then_inc(cc, 1)
            eng.wait_ge(cc, T + 1)
            for c in range(32):
                eng.dma_start(
                    out=esm_sb[:, c:c + 1],
                    in_=AP(e_all, 128 * c, [[1, 128], [1, 1]]),
                ).then_inc(g16, 16)
            eng.wait_ge(dv, DV_OUT)
            for c in range(32):
                eng.dma_start(
                    out=AP(out_ext, 128 * c, [[1, 128], [1, 1]]),
                    in_=osb[:, c:c + 1],
                ).then_inc(g16, 16)

        @block.tensor
        def _(eng):
            eng.wait_ge(gc, 4)
            eng.wait_ge(g16, G_GATH)
            for i in range(64):                      # i = 16*cp + k
                cp, k = i // 16, i % 16
                if i >= 8:
                    eng.wait_ge(ac, AC_COPY(i - 8))  # bank free after copy
                src_col = (1024 * cp + 128 * k) if k < 8 else (4096 + 1024 * cp + 128 * (k - 8))
                eng.transpose(
                    psum[i % 8][:, 0:128],
                    big_sb[:, src_col:src_col + 128],
                    id_sb[:, :],
                ).then_inc(pe, 1)
            eng.wait_ge(ld, LD_WIH)
            eng.wait_ge(ac, AC_COPY(63))             # xs_T fully written
            for j in range(NJ):
                for c in range(NK):
                    inst = eng.matmul(
                        psum[j][:, 0:512],
                        big_sb[:, 1024 * c + 128 * j:1024 * c + 128 * j + 128],
                        xs_T[:, 512 * c:512 * c + 512],
                        start=(c == 0), stop=(c == NK - 1),
                    )
                    if c == NK - 1:
                        inst.then_inc(pe, 1)
            eng.wait_ge(dv, DV_A)                    # A-adds done: psum 0/1 free
            for t in range(1, T):
                tq, rq = t % 2, (t - 1) % 2
                eng.wait_ge(g16, G_IN(t - 1))
                for j in range(NJ):
                    for k in range(NK):
                        inst = eng.matmul(
                            psum[tq][:, j:j + 1],
                            whh_sb[:, 1024 * k + 128 * j:1024 * k + 128 * j + 128],
                            h_all_sb[:, 16 * rq + k:16 * rq + k + 1],
                            start=(k == 0), stop=(k == NK - 1),
                        )
                        if j == NJ - 1 and k == NK - 1:
                            inst.then_inc(pe, 1)
            eng.wait_ge(g16, G_IN(T - 1))
            eng.wait_ge(ld, LD_WFC)
            fq = (T - 1) % 2
            for j in range(4):
                for k in range(NK):
                    inst = eng.matmul(
                        psum[2][:, j:j + 1],
                        wfc_sb[:, 512 * k + 128 * j:512 * k + 128 * j + 128],
                        h_all_sb[:, 16 * fq + k:16 * fq + k + 1],
                        start=(k == 0), stop=(k == NK - 1),
                    )
                    if j == 3 and k == NK - 1:
                        inst.then_inc(pe, 1)
            eng.wait_ge(dv, DV_RED)
            eng.matmul(psum[2][0:1, 8:9], ones_p[:, :], red_sb[:, :],
                       start=True, stop=True).then_inc(pe, 1)
            eng.wait_ge(dv, DV_RECIP)
            eng.matmul(psum[2][:, 9:10], ones_f[:, :], rs_sb[:, :],
                       start=True, stop=True).then_inc(pe, 1)

        @block.vector
        def _(eng):
            for j in range(NJ):
                eng.wait_ge(pe, PE_APRE(j))
                eng.tensor_scalar_add(
                    AP(A_sb, j, [[NJ * T, 128], [NJ, T]]),
                    psum[j][:, 0:512],
                    b_sb[:, j:j + 1],
                ).then_inc(dv, 1)
            eng.wait_ge(dv, DV_A)          # A_sb writes retired before self-reads
            # step 0: gates come straight from A (h=0), c_in=0 -> c = sig(i)*tanh(g)
            eng.wait_ge(ac, AC_A1(0))
            eng.tensor_mul(cq_sb[:, 0:2], nl_sb[:, 0:2], nl_sb[:, 4:6]).then_inc(dv, 1)
            eng.wait_ge(ac, AC_A2(0))
            eng.tensor_mul(h_new_sb[:, 0:2], nl_sb[:, 6:8], tc_sb[:, 0:2]).then_inc(dv, 1)
            for t in range(1, T):
                tq = t % 2
                eng.wait_ge(pe, PE_STEP(t))
                eng.tensor_add(
                    gates_sb[:, 8 * tq:8 * tq + 8],
                    psum[tq][:, 0:8],
                    A_sb[:, 8 * t:8 * t + 8],
                ).then_inc(dv, 1)
                eng.wait_ge(ac, AC_A1(t))
                eng.tensor_mul(
                    tmp_sb[:, 4 * tq:4 * tq + 2],
                    nl_sb[:, 8 * tq + 2:8 * tq + 4],
                    AP(xs_T, t, [[NK * T, 128], [512, 2]]),   # c_in = x_t (features 0:256)
                ).then_inc(vw, 1)
                eng.tensor_mul(
                    tmp_sb[:, 4 * tq + 2:4 * tq + 4],
                    nl_sb[:, 8 * tq:8 * tq + 2],
                    nl_sb[:, 8 * tq + 4:8 * tq + 6],
                ).then_inc(vw, 1)
                eng.wait_ge(vw, 2 * t)
                eng.tensor_add(
                    cq_sb[:, 2 * tq:2 * tq + 2],
                    tmp_sb[:, 4 * tq:4 * tq + 2],
                    tmp_sb[:, 4 * tq + 2:4 * tq + 4],
                ).then_inc(dv, 1)
                eng.wait_ge(ac, AC_A2(t))
                eng.tensor_mul(
                    h_new_sb[:, 2 * tq:2 * tq + 2],
                    nl_sb[:, 8 * tq + 6:8 * tq + 8],
                    tc_sb[:, 2 * tq:2 * tq + 2],
                ).then_inc(dv, 1)
            eng.wait_ge(pe, PE_FC)
            eng.tensor_add(fcl_sb[:, :], psum[2][:, 0:4], bfc_sb[:, :]).then_inc(dv, 1)
            eng.wait_ge(g16, G_ESM)
            eng.tensor_reduce(red_sb[:, :], esm_sb[:, :],
                              axis=mybir.AxisListType.X, op=mybir.AluOpType.add).then_inc(dv, 1)
            eng.wait_ge(pe, PE_SUM)
            eng.reciprocal(rs_sb[:, :], psum[2][0:1, 8:9]).then_inc(dv, 1)
            eng.wait_ge(pe, PE_BC)
            eng.tensor_scalar_mul(osb[:, :], esm_sb[:, :], psum[2][:, 9:10]).then_inc(dv, 1)

        @block.scalar
        def _(eng):
            for i in range(64):
                cp, k = i // 16, i % 16
                eng.wait_ge(pe, i + 1)
                # transpose out free index i maps to t = 4i + cp -> stride-4 scatter
                eng.activation(
                    AP(xs_T, 512 * k + cp, [[NK * T, 128], [4, 128]]),
                    psum[i % 8][:, 0:128], Cpy,
                ).then_inc(ac, 1)
            for t in range(T):
                tq = t % 2
                if t == 0:
                    eng.wait_ge(dv, DV_A)
                    g_ap = A_sb
                    base = 0
                else:
                    eng.wait_ge(dv, DV_GATES(t))
                    g_ap = gates_sb
                    base = 8 * tq
                eng.activation(nl_sb[:, 8 * tq:8 * tq + 4], g_ap[:, base:base + 4], Sig)
                eng.activation(nl_sb[:, 8 * tq + 4:8 * tq + 6], g_ap[:, base + 4:base + 6], Tnh)
                eng.activation(nl_sb[:, 8 * tq + 6:8 * tq + 8], g_ap[:, base + 6:base + 8], Sig).then_inc(ac, 1)
                eng.wait_ge(dv, DV_C0 if t == 0 else DV_C(t))
                eng.activation(tc_sb[:, 2 * tq:2 * tq + 2], cq_sb[:, 2 * tq:2 * tq + 2], Tnh).then_inc(ac, 1)
            eng.wait_ge(dv, DV_FC)
            eng.activation(exp_sb[:, :], fcl_sb[:, :], ExpF).then_inc(ac, 1)

    return nc


LAST_EXEC_NS = None


def kernel(**inputs):
    global LAST_EXEC_NS
    from concourse import bass_utils

    if "nc" not in _CACHE:
        _CACHE["nc"] = _build()
    nc = _CACHE["nc"]

    in_maps = _prep_in_maps(inputs)
    trace = bool(int(os.environ.get("KERNEL_TRACE", "0")))
    if trace:
        try:
            res = bass_utils.run_bass_kernel_spmd(nc, in_maps, list(range(M)), trace=True)
        except Exception:
            res = bass_utils.run_bass_kernel_spmd(nc, in_maps, list(range(M)), trace=False)
    else:
        res = bass_utils.run_bass_kernel_spmd(nc, in_maps, list(range(M)), trace=False)
    LAST_EXEC_NS = getattr(res, "exec_time_ns", None)
    out = np.asarray(res.results[0]["out"], dtype=np.float32)
    return out.reshape(1, V)


# revision 32
# speedup vs baseline: 3.3220x; 1.0012x over previous
"""LSTM-like policy net on 8 Trainium2 cores, tensor-parallel over the gate dim.

Per-core shard m owns gate rows [256m, 256m+256) of each gate (i,f,g,o),
h elements [256m, 256m+256), and fc rows [512m, 512m+512).
Per step: local W_hh @ h matvec (128 accumulating matmuls), nonlinearities,
AllGather of the 256-element h shard. x-path (embeddings @ W_ih) is
precomputed for all 512 steps in one parallel pass.
"""

import os
import sys

import ml_dtypes
import numpy as np

if "/opt/trn_rl_repo" not in sys.path:
    sys.path.insert(0, "/opt/trn_rl_repo")

T = 512          # sequence length
D = 2048         # input feature dim (2 x 1024 embeddings)
H = 2048         # hidden dim
L = 1024         # local gate rows per core (4 gates x 256)
V = 4096         # fc output dim
M = 8            # cores
NK = 16          # 128-chunks over D/H
NJ = 8           # 128-chunks over L

_CACHE = {}


def _h_perm():
    # h_all flat index u = 256*q + 2*p + jl  ->  h element 256*q + 128*jl + p
    u = np.arange(H)
    q, r = u // 256, u % 256
    return 256 * q + 128 * (r % 2) + r // 2


def _contract_perm():
    # whh_sb row v = 128*k + kk multiplies rhs element h_all[16*kk + k]
    v = np.arange(H)
    k, kk = v // 128, v % 128
    return _h_perm()[16 * kk + k]


def _prep_in_maps(inputs):
    gz = np.ascontiguousarray(np.asarray(inputs["guesses"]).astype(np.int32).ravel())
    fb = np.ascontiguousarray(np.asarray(inputs["feedbacks"]).astype(np.int32).ravel())
    ge = np.asarray(inputs["guess_embed"], dtype=np.float32)
    fe = np.asarray(inputs["feedback_embed"], dtype=np.float32)
    W_ih = np.asarray(inputs["W_ih"], dtype=np.float32)
    W_hh = np.asarray(inputs["W_hh"], dtype=np.float32)
    bias = (np.asarray(inputs["b_ih"], dtype=np.float32)
            + np.asarray(inputs["b_hh"], dtype=np.float32))
    W_fc = np.asarray(inputs["W_fc"], dtype=np.float32)
    b_fc = np.asarray(inputs["b_fc"], dtype=np.float32)

    cperm = _contract_perm()
    in_maps = []
    for m in range(M):
        # local gate row r = 128*jj + p  ->  global row 2048*(jj//2) + 256*m + 128*(jj%2) + p
        jj = np.arange(NJ)
        rows = (2048 * (jj // 2)[:, None] + 256 * m + 128 * (jj % 2)[:, None]
                + np.arange(128)[None, :]).ravel()
        Wih_sh = W_ih[rows]            # [1024, 2048]
        Whh_sh = W_hh[rows]            # [1024, 2048]
        b_sh = np.ascontiguousarray(bias[rows])

        # x-space feature permutation: own embedding half rolled so this
        # core's 256 c_in features land at positions 0:256
        own = ge if m < 4 else fe
        oth = fe if m < 4 else ge
        own_base = 0 if m < 4 else 1024
        roll = (np.arange(1024) + 256 * (m % 4)) % 1024
        perm = np.concatenate([own_base + roll, (1024 - own_base) + np.arange(1024)])

        in_maps.append({
            "idx_a": gz if m < 4 else fb,
            "idx_b": fb if m < 4 else gz,
            "tab_a": np.ascontiguousarray(own[:, roll]),
            "tab_b": np.ascontiguousarray(oth),
            "wih_t": np.ascontiguousarray(Wih_sh[:, perm].T),      # [2048, 1024]
            "whh_t": np.ascontiguousarray(Whh_sh[:, cperm].T).astype(ml_dtypes.float8_e4m3fn),
            # reordered so a contiguous [[8,128],[1,8]] load puts b_sh[128j+p] at (p,j)
            "bias": np.ascontiguousarray(b_sh.reshape(8, 128).T.ravel()),
            "wfc_t": np.ascontiguousarray(W_fc[512 * m:512 * m + 512][:, cperm].T).astype(ml_dtypes.bfloat16),
            "bfc": np.ascontiguousarray(
                b_fc[512 * m:512 * m + 512].reshape(4, 128).T.ravel()),
        })
    return in_maps


def _build():
    from concourse import bass, mybir

    f32 = mybir.dt.float32
    bf16 = mybir.dt.bfloat16
    fp8 = mybir.dt.float8e4
    i32 = mybir.dt.int32
    Sig = mybir.ActivationFunctionType.Sigmoid
    Tnh = mybir.ActivationFunctionType.Tanh
    ExpF = mybir.ActivationFunctionType.Exp
    Cpy = mybir.ActivationFunctionType.Copy
    AP = bass.AP

    nc = bass.Bass(target_bir_lowering=False, debug=False)

    idx_a = nc.declare_dram_parameter("idx_a", [T], i32, isOutput=False)
    idx_b = nc.declare_dram_parameter("idx_b", [T], i32, isOutput=False)
    tab_a = nc.declare_dram_parameter("tab_a", [4097, 1024], f32, isOutput=False)
    tab_b = nc.declare_dram_parameter("tab_b", [4097, 1024], f32, isOutput=False)
    wih_t = nc.declare_dram_parameter("wih_t", [D, L], f32, isOutput=False)
    whh_t = nc.declare_dram_parameter("whh_t", [H, L], fp8, isOutput=False)
    bias_d = nc.declare_dram_parameter("bias", [L], f32, isOutput=False)
    wfc_t = nc.declare_dram_parameter("wfc_t", [H, 512], bf16, isOutput=False)
    bfc_d = nc.declare_dram_parameter("bfc", [512], f32, isOutput=False)
    out_ext = nc.declare_dram_parameter("out", [V], f32, isOutput=True)

    h_loc = nc.dram_tensor("h_loc", [256], bf16)
    h_all = nc.dram_tensor("h_all", [H], bf16, addr_space="Shared")
    e_loc = nc.dram_tensor("e_loc", [512], f32)
    e_all = nc.dram_tensor("e_all", [V], f32, addr_space="Shared")

    whh_sb = nc.alloc_sbuf_tensor("whh_sb", [128, H * NJ], fp8)     # 16KB/part
    wfc_sb = nc.alloc_sbuf_tensor("wfc_sb", [128, 8192], bf16)      # 16KB/part
    big_sb = nc.alloc_sbuf_tensor("big_sb", [128, 16384], f32)      # gathers->wih->wfc
    xs_T = nc.alloc_sbuf_tensor("xs_T", [128, NK * T], f32)         # 32KB/part
    A_sb = nc.alloc_sbuf_tensor("A_sb", [128, NJ * T], f32)         # 16KB/part
    id_sb = nc.alloc_sbuf_tensor("id_sb", [128, 128], f32)
    ones_p = nc.alloc_sbuf_tensor("ones_p", [128, 1], f32)
    ones_f = nc.alloc_sbuf_tensor("ones_f", [1, 128], f32)
    b_sb = nc.alloc_sbuf_tensor("b_sb", [128, NJ], f32)
    bfc_sb = nc.alloc_sbuf_tensor("bfc_sb", [128, 4], f32)
    idxa_sb = nc.alloc_sbuf_tensor("idxa_sb", [128, 4], i32)
    idxb_sb = nc.alloc_sbuf_tensor("idxb_sb", [128, 4], i32)
    h_all_sb = nc.alloc_sbuf_tensor("h_all_sb", [128, 32], bf16)    # 2 parity halves
    h_new_sb = nc.alloc_sbuf_tensor("h_new_sb", [128, 4], bf16)
    gates_sb = nc.alloc_sbuf_tensor("gates_sb", [128, 16], f32)
    nl_sb = nc.alloc_sbuf_tensor("nl_sb", [128, 16], f32)
    tmp_sb = nc.alloc_sbuf_tensor("tmp_sb", [128, 8], f32)
    cq_sb = nc.alloc_sbuf_tensor("cq_sb", [128, 4], f32)
    tc_sb = nc.alloc_sbuf_tensor("tc_sb", [128, 4], f32)
    fcl_sb = nc.alloc_sbuf_tensor("fcl_sb", [128, 4], f32)
    exp_sb = nc.alloc_sbuf_tensor("exp_sb", [128, 4], f32)
    esm_sb = nc.alloc_sbuf_tensor("esm_sb", [128, 32], f32)
    osb = nc.alloc_sbuf_tensor("osb", [128, 32], f32)
    red_sb = nc.alloc_sbuf_tensor("red_sb", [128, 1], f32)
    rs_sb = nc.alloc_sbuf_tensor("rs_sb", [1, 1], f32)

    psum = [nc.alloc_psum_tensor(f"ps{j}", [128, 512], f32) for j in range(8)]

    cores = list(range(M))

    # --- static semaphore schedule ---------------------------------------
    PE_TRANS = 64                      # after 64 transposes
    PE_APRE = lambda j: PE_TRANS + j + 1      # after A-precompute col-block j
    PE_STEP = lambda t: 72 + t                # after step-t matvecs (t>=1)
    PE_FC = 584
    PE_SUM = 585
    PE_BC = 586
    G_GATH = 128                               # 8 gathers done
    G_OUT = lambda t: G_GATH + 32 * t + 16     # h_loc out-dma of step t
    G_IN = lambda t: G_GATH + 32 * t + 32      # h_all in-dma of step t
    G_ELOC = G_IN(T - 1) + 4 * 16
    G_ESM = G_ELOC + 32 * 16
    DV_A = 8
    DV_C0 = 9
    DV_H0 = 10
    DV_GATES = lambda t: 3 * t + 8
    DV_C = lambda t: 3 * t + 9
    DV_H = lambda t: 3 * t + 10
    DV_FC = DV_H(T - 1) + 1            # 1544
    DV_RED = DV_FC + 1
    DV_RECIP = DV_FC + 2
    DV_OUT = DV_FC + 3
    AC_COPY = lambda i: i + 1
    AC_A1 = lambda t: 2 * t + 65
    AC_A2 = lambda t: 2 * t + 66
    AC_EXP = AC_A2(T - 1) + 1          # 1089
    LD_WHH, LD_BIAS, LD_BFC, LD_WFC, LD_WIH = 16, 32, 48, 64, 80
    LDI_IDX = 32

    with (
        nc.Block() as block,
        nc.semaphore("ld") as ld,
        nc.semaphore("ldi") as ldi,
        nc.semaphore("gc") as gc,
        nc.semaphore("g16") as g16,
        nc.semaphore("cc") as cc,
        nc.semaphore("pe") as pe,
        nc.semaphore("dv") as dv,
        nc.semaphore("ac") as ac,
        nc.semaphore("vw") as vw,
    ):

        @block.sync
        def _(eng):
            # (p, c) = idx[4p + c]: gather tile c holds timestep t = 4p + c at partition p
            eng.dma_start(out=idxa_sb[:, :], in_=AP(idx_a, 0, [[4, 128], [1, 4]])).then_inc(ldi, 16)
            eng.dma_start(out=idxb_sb[:, :], in_=AP(idx_b, 0, [[4, 128], [1, 4]])).then_inc(ldi, 16)
            eng.dma_start(
                out=AP(whh_sb, 0, [[16384, 128], [1024, 16], [1, 1024]]),
                in_=AP(whh_t, 0, [[1024, 128], [131072, 16], [1, 1024]]),
            ).then_inc(ld, 16)
            eng.dma_start(out=b_sb[:, :], in_=AP(bias_d, 0, [[8, 128], [1, 8]])).then_inc(ld, 16)
            eng.dma_start(out=bfc_sb[:, :], in_=AP(bfc_d, 0, [[4, 128], [1, 4]])).then_inc(ld, 16)
            eng.dma_start(
                out=AP(wfc_sb, 0, [[8192, 128], [512, 16], [1, 512]]),
                in_=AP(wfc_t, 0, [[512, 128], [65536, 16], [1, 512]]),
            ).then_inc(ld, 16)
            eng.wait_ge(pe, PE_TRANS)      # transposes done reading big_sb
            eng.dma_start(
                out=AP(big_sb, 0, [[16384, 128], [1024, 16], [1, 1024]]),
                in_=AP(wih_t, 0, [[1024, 128], [131072, 16], [1, 1024]]),
            ).then_inc(ld, 16)

        @block.gpsimd
        def _(eng):
            eng.memset(id_sb[:, :], 1.0).then_inc(gc, 1)
            eng.memset(ones_p[:, :], 1.0).then_inc(gc, 1)
            eng.memset(ones_f[:, :], 1.0).then_inc(gc, 1)
            eng.wait_ge(gc, 3)
            eng.affine_select(
                id_sb[:, :], id_sb[:, :], pattern=[[1, 128]],
                compare_op=mybir.AluOpType.is_equal, fill=0.0,
                base=0, channel_multiplier=-1,
            ).then_inc(gc, 1)
            eng.wait_ge(ldi, LDI_IDX)
            for c in range(4):
                eng.indirect_dma_start(
                    out=big_sb[:, 1024 * c:1024 * c + 1024], out_offset=None,
                    in_=tab_a[:, :],
                    in_offset=bass.IndirectOffsetOnAxis(ap=idxa_sb[:, c:c + 1], axis=0),
                ).then_inc(g16, 16)
            for c in range(4):
                eng.indirect_dma_start(
                    out=big_sb[:, 4096 + 1024 * c:4096 + 1024 * c + 1024], out_offset=None,
                    in_=tab_b[:, :],
                    in_offset=bass.IndirectOffsetOnAxis(ap=idxb_sb[:, c:c + 1], axis=0),
                ).then_inc(g16, 16)
            for t in range(T):
                tq = t % 2
                eng.wait_ge(dv, DV_H0 if t == 0 else DV_H(t))
                eng.dma_start(
                    out=AP(h_loc, 0, [[2, 128], [1, 2]]),
                    in_=h_new_sb[:, 2 * tq:2 * tq + 2],
                ).then_inc(g16, 16)
                eng.wait_ge(g16, G_OUT(t))
                eng.collective_compute(
                    "AllGather", mybir.AluOpType.bypass,
                    replica_groups=[cores],
                    ins=[h_loc[:]], outs=[h_all[:]],
                ).then_inc(cc, 1)
                eng.wait_ge(cc, t + 1)
                eng.dma_start(
                    out=h_all_sb[:, 16 * tq:16 * tq + 16],
                    in_=AP(h_all, 0, [[16, 128], [1, 16]]),
                ).then_inc(g16, 16)
            eng.wait_ge(ac, AC_EXP)
            for j in range(4):
                eng.dma_start(
                    out=AP(e_loc, 128 * j, [[1, 128], [1, 1]]),
                    in_=exp_sb[:, j:j + 1],
                ).then_inc(g16, 16)
            eng.wait_ge(g16, G_ELOC)
            eng.collective_compute(
                "AllGather", mybir.AluOpType.bypass,
                replica_groups=[cores],
                ins=[e_loc[:]], outs=[e_all[:]],
            ).then_inc(cc, 1)
            eng.wait_ge(cc, T + 1)
            for c in range(32):
                eng.dma_start(
                    out=esm_sb[:, c:c + 1],
                    in_=AP(e_all, 128 * c, [[1, 128], [1, 1]]),
                ).then_inc(g16, 16)
            eng.wait_ge(dv, DV_OUT)
            for c in range(32):
                eng.dma_start(
                    out=AP(out_ext, 128 * c, [[1, 128], [1, 1]]),
                    in_=osb[:, c:c + 1],
                ).then_inc(g16, 16)

        @block.tensor
        def _(eng):
            eng.wait_ge(gc, 4)
            eng.wait_ge(g16, G_GATH)
            for i in range(64):                      # i = 16*cp + k
                cp, k = i // 16, i % 16
                if i >= 8:
                    eng.wait_ge(ac, AC_COPY(i - 8))  # bank free after copy
                src_col = (1024 * cp + 128 * k) if k < 8 else (4096 + 1024 * cp + 128 * (k - 8))
                eng.transpose(
                    psum[i % 8][:, 0:128],
                    big_sb[:, src_col:src_col + 128],
                    id_sb[:, :],
                ).then_inc(pe, 1)
            eng.wait_ge(ld, LD_WIH)
            eng.wait_ge(ac, AC_COPY(63))             # xs_T fully written
            for j in range(NJ):
                for c in range(NK):
                    inst = eng.matmul(
                        psum[j][:, 0:512],
                        big_sb[:, 1024 * c + 128 * j:1024 * c + 128 * j + 128],
                        xs_T[:, 512 * c:512 * c + 512],
                        start=(c == 0), stop=(c == NK - 1),
                    )
                    if c == NK - 1:
                        inst.then_inc(pe, 1)
            eng.wait_ge(dv, DV_A)                    # A-adds done: psum 0/1 free
            for t in range(1, T):
                tq, rq = t % 2, (t - 1) % 2
                eng.wait_ge(g16, G_IN(t - 1))
                for j in range(NJ):
                    for k in range(NK):
                        inst = eng.matmul(
                            psum[tq][:, j:j + 1],
                            whh_sb[:, 1024 * k + 128 * j:1024 * k + 128 * j + 128],
                            h_all_sb[:, 16 * rq + k:16 * rq + k + 1],
                            start=(k == 0), stop=(k == NK - 1),
                        )
                        if j == NJ - 1 and k == NK - 1:
                            inst.then_inc(pe, 1)
            eng.wait_ge(g16, G_IN(T - 1))
            eng.wait_ge(ld, LD_WFC)
            fq = (T - 1) % 2
            for j in range(4):
                for k in range(NK):
                    inst = eng.matmul(
                        psum[2][:, j:j + 1],
                        wfc_sb[:, 512 * k + 128 * j:512 * k + 128 * j + 128],
                        h_all_sb[:, 16 * fq + k:16 * fq + k + 1],
                        start=(k == 0), stop=(k == NK - 1),
                    )
                    if j == 3 and k == NK - 1:
                        inst.then_inc(pe, 1)
            eng.wait_ge(dv, DV_RED)
            eng.matmul(psum[2][0:1, 8:9], ones_p[:, :], red_sb[:, :],
                       start=True, stop=True).then_inc(pe, 1)
            eng.wait_ge(dv, DV_RECIP)
            eng.matmul(psum[2][:, 9:10], ones_f[:, :], rs_sb[:, :],
                       start=True, stop=True).then_inc(pe, 1)

        @block.vector
        def _(eng):
            for j in range(NJ):
                eng.wait_ge(pe, PE_APRE(j))
                eng.tensor_scalar_add(
                    AP(A_sb, j, [[NJ * T, 128], [NJ, T]]),
                    psum[j][:, 0:512],
                    b_sb[:, j:j + 1],
                ).then_inc(dv, 1)
            eng.wait_ge(dv, DV_A)          # A_sb writes retired before self-reads
            # step 0: gates come straight from A (h=0), c_in=0 -> c = sig(i)*tanh(g)
            eng.wait_ge(ac, AC_A1(0))
            eng.tensor_mul(cq_sb[:, 0:2], nl_sb[:, 0:2], nl_sb[:, 4:6]).then_inc(dv, 1)
            eng.wait_ge(ac, AC_A2(0))
            eng.tensor_mul(h_new_sb[:, 0:2], nl_sb[:, 6:8], tc_sb[:, 0:2]).then_inc(dv, 1)
            for t in range(1, T):
                tq = t % 2
                eng.wait_ge(pe, PE_STEP(t))
                eng.tensor_add(
                    gates_sb[:, 8 * tq:8 * tq + 8],
                    psum[tq][:, 0:8],
                    A_sb[:, 8 * t:8 * t + 8],
                ).then_inc(dv, 1)
                eng.wait_ge(ac, AC_A1(t))
                eng.tensor_mul(
                    tmp_sb[:, 4 * tq:4 * tq + 2],
                    nl_sb[:, 8 * tq + 2:8 * tq + 4],
                    AP(xs_T, t, [[NK * T, 128], [512, 2]]),   # c_in = x_t (features 0:256)
                ).then_inc(vw, 1)
                eng.tensor_mul(
                    tmp_sb[:, 4 * tq + 2:4 * tq + 4],
                    nl_sb[:, 8 * tq:8 * tq + 2],
                    nl_sb[:, 8 * tq + 4:8 * tq + 6],
                ).then_inc(vw, 1)
                eng.wait_ge(vw, 2 * t)
                eng.tensor_add(
                    cq_sb[:, 2 * tq:2 * tq + 2],
                    tmp_sb[:, 4 * tq:4 * tq + 2],
                    tmp_sb[:, 4 * tq + 2:4 * tq + 4],
                ).then_inc(dv, 1)
                eng.wait_ge(ac, AC_A2(t))
                eng.tensor_mul(
                    h_new_sb[:, 2 * tq:2 * tq + 2],
                    nl_sb[:, 8 * tq + 6:8 * tq + 8],
                    tc_sb[:, 2 * tq:2 * tq + 2],
                ).then_inc(dv, 1)
            eng.wait_ge(pe, PE_FC)
            eng.tensor_add(fcl_sb[:, :], psum[2][:, 0:4], bfc_sb[:, :]).then_inc(dv, 1)
            eng.wait_ge(g16, G_ESM)
            eng.tensor_reduce(red_sb[:, :], esm_sb[:, :],
                              axis=mybir.AxisListType.X, op=mybir.AluOpType.add).then_inc(dv, 1)
            eng.wait_ge(pe, PE_SUM)
            eng.reciprocal(rs_sb[:, :], psum[2][0:1, 8:9]).then_inc(dv, 1)
            eng.wait_ge(pe, PE_BC)
            eng.tensor_scalar_mul(osb[:, :], esm_sb[:, :], psum[2][:, 9:10]).then_inc(dv, 1)

        @block.scalar
        def _(eng):
            for i in range(64):
                cp, k = i // 16, i % 16
                eng.wait_ge(pe, i + 1)
                # transpose out free index i maps to t = 4i + cp -> stride-4 scatter
                eng.activation(
                    AP(xs_T, 512 * k + cp, [[NK * T, 128], [4, 128]]),
                    psum[i % 8][:, 0:128], Cpy,
                ).then_inc(ac, 1)
            for t in range(T):
                tq = t % 2
                if t == 0:
                    eng.wait_ge(dv, DV_A)
                    g_ap = A_sb
                    base = 0
                else:
                    eng.wait_ge(dv, DV_GATES(t))
                    g_ap = gates_sb
                    base = 8 * tq
                eng.activation(nl_sb[:, 8 * tq:8 * tq + 4], g_ap[:, base:base + 4], Sig)
                eng.activation(nl_sb[:, 8 * tq + 4:8 * tq + 6], g_ap[:, base + 4:base + 6], Tnh)
                eng.activation(nl_sb[:, 8 * tq + 6:8 * tq + 8], g_ap[:, base + 6:base + 8], Sig).then_inc(ac, 1)
                eng.wait_ge(dv, DV_C0 if t == 0 else DV_C(t))
                eng.activation(tc_sb[:, 2 * tq:2 * tq + 2], cq_sb[:, 2 * tq:2 * tq + 2], Tnh).then_inc(ac, 1)
            eng.wait_ge(dv, DV_FC)
            eng.activation(exp_sb[:, :], fcl_sb[:, :], ExpF).then_inc(ac, 1)

    return nc


LAST_EXEC_NS = None


def kernel(**inputs):
    global LAST_EXEC_NS
    from concourse import bass_utils

    if "nc" not in _CACHE:
        _CACHE["nc"] = _build()
    nc = _CACHE["nc"]

    in_maps = _prep_in_maps(inputs)
    trace = bool(int(os.environ.get("KERNEL_TRACE", "0")))
    if trace:
        try:
            res = bass_utils.run_bass_kernel_spmd(nc, in_maps, list(range(M)), trace=True)
        except Exception:
            res = bass_utils.run_bass_kernel_spmd(nc, in_maps, list(range(M)), trace=False)
    else:
        res = bass_utils.run_bass_kernel_spmd(nc, in_maps, list(range(M)), trace=False)
    LAST_EXEC_NS = getattr(res, "exec_time_ns", None)
    out = np.asarray(res.results[0]["out"], dtype=np.float32)
    return out.reshape(1, V)


# revision 33
# speedup vs baseline: 3.4105x; 1.0266x over previous
"""LSTM-like policy net on 8 Trainium2 cores, tensor-parallel over the gate dim.

Per-core shard m owns gate rows [256m, 256m+256) of each gate (i,f,g,o),
h elements [256m, 256m+256), and fc rows [512m, 512m+512).
Per step: local W_hh @ h matvec (128 accumulating matmuls), nonlinearities,
AllGather of the 256-element h shard. x-path (embeddings @ W_ih) is
precomputed for all 512 steps in one parallel pass.
"""

import os
import sys

import ml_dtypes
import numpy as np

if "/opt/trn_rl_repo" not in sys.path:
    sys.path.insert(0, "/opt/trn_rl_repo")

T = 512          # sequence length
D = 2048         # input feature dim (2 x 1024 embeddings)
H = 2048         # hidden dim
L = 1024         # local gate rows per core (4 gates x 256)
V = 4096         # fc output dim
M = 8            # cores
NK = 16          # 128-chunks over D/H
NJ = 8           # 128-chunks over L

_CACHE = {}


def _h_perm():
    # h_all flat index u = 256*q + 2*p + jl  ->  h element 256*q + 128*jl + p
    u = np.arange(H)
    q, r = u // 256, u % 256
    return 256 * q + 128 * (r % 2) + r // 2


def _contract_perm():
    # whh_sb row v = 128*k + kk multiplies rhs element h_all[16*kk + k]
    v = np.arange(H)
    k, kk = v // 128, v % 128
    return _h_perm()[16 * kk + k]


def _prep_in_maps(inputs):
    gz = np.ascontiguousarray(np.asarray(inputs["guesses"]).astype(np.int32).ravel())
    fb = np.ascontiguousarray(np.asarray(inputs["feedbacks"]).astype(np.int32).ravel())
    ge = np.asarray(inputs["guess_embed"], dtype=np.float32)
    fe = np.asarray(inputs["feedback_embed"], dtype=np.float32)
    W_ih = np.asarray(inputs["W_ih"], dtype=np.float32)
    W_hh = np.asarray(inputs["W_hh"], dtype=np.float32)
    bias = (np.asarray(inputs["b_ih"], dtype=np.float32)
            + np.asarray(inputs["b_hh"], dtype=np.float32))
    W_fc = np.asarray(inputs["W_fc"], dtype=np.float32)
    b_fc = np.asarray(inputs["b_fc"], dtype=np.float32)

    cperm = _contract_perm()
    in_maps = []
    for m in range(M):
        # local gate row r = 128*jj + p  ->  global row 2048*(jj//2) + 256*m + 128*(jj%2) + p
        jj = np.arange(NJ)
        rows = (2048 * (jj // 2)[:, None] + 256 * m + 128 * (jj % 2)[:, None]
                + np.arange(128)[None, :]).ravel()
        Wih_sh = W_ih[rows]            # [1024, 2048]
        Whh_sh = W_hh[rows]            # [1024, 2048]
        b_sh = np.ascontiguousarray(bias[rows])

        # x-space feature permutation: own embedding half rolled so this
        # core's 256 c_in features land at positions 0:256
        own = ge if m < 4 else fe
        oth = fe if m < 4 else ge
        own_base = 0 if m < 4 else 1024
        roll = (np.arange(1024) + 256 * (m % 4)) % 1024
        perm = np.concatenate([own_base + roll, (1024 - own_base) + np.arange(1024)])

        in_maps.append({
            "idx_a": gz if m < 4 else fb,
            "idx_b": fb if m < 4 else gz,
            "tab_a": np.ascontiguousarray(own[:, roll]),
            "tab_b": np.ascontiguousarray(oth),
            "wih_t": np.ascontiguousarray(Wih_sh[:, perm].T),      # [2048, 1024]
            "whh_t": np.ascontiguousarray(Whh_sh[:, cperm].T).astype(ml_dtypes.bfloat16),
            # reordered so a contiguous [[8,128],[1,8]] load puts b_sh[128j+p] at (p,j)
            "bias": np.ascontiguousarray(b_sh.reshape(8, 128).T.ravel()),
            "wfc_t": np.ascontiguousarray(W_fc[512 * m:512 * m + 512][:, cperm].T).astype(ml_dtypes.bfloat16),
            "bfc": np.ascontiguousarray(
                b_fc[512 * m:512 * m + 512].reshape(4, 128).T.ravel()),
        })
    return in_maps


def _build():
    from concourse import bass, mybir

    f32 = mybir.dt.float32
    bf16 = mybir.dt.bfloat16
    i32 = mybir.dt.int32
    Sig = mybir.ActivationFunctionType.Sigmoid
    Tnh = mybir.ActivationFunctionType.Tanh
    ExpF = mybir.ActivationFunctionType.Exp
    Cpy = mybir.ActivationFunctionType.Copy
    AP = bass.AP

    nc = bass.Bass(target_bir_lowering=False, debug=False)

    idx_a = nc.declare_dram_parameter("idx_a", [T], i32, isOutput=False)
    idx_b = nc.declare_dram_parameter("idx_b", [T], i32, isOutput=False)
    tab_a = nc.declare_dram_parameter("tab_a", [4097, 1024], f32, isOutput=False)
    tab_b = nc.declare_dram_parameter("tab_b", [4097, 1024], f32, isOutput=False)
    wih_t = nc.declare_dram_parameter("wih_t", [D, L], f32, isOutput=False)
    whh_t = nc.declare_dram_parameter("whh_t", [H, L], bf16, isOutput=False)
    bias_d = nc.declare_dram_parameter("bias", [L], f32, isOutput=False)
    wfc_t = nc.declare_dram_parameter("wfc_t", [H, 512], bf16, isOutput=False)
    bfc_d = nc.declare_dram_parameter("bfc", [512], f32, isOutput=False)
    out_ext = nc.declare_dram_parameter("out", [V], f32, isOutput=True)

    h_loc = nc.dram_tensor("h_loc", [256], bf16)
    h_all = nc.dram_tensor("h_all", [H], bf16, addr_space="Shared")
    e_loc = nc.dram_tensor("e_loc", [512], f32)
    e_all = nc.dram_tensor("e_all", [V], f32, addr_space="Shared")

    whh_sb = nc.alloc_sbuf_tensor("whh_sb", [128, H * NJ], bf16)    # 32KB/part
    wfc_sb = nc.alloc_sbuf_tensor("wfc_sb", [128, 8192], bf16)      # 16KB/part
    big_sb = nc.alloc_sbuf_tensor("big_sb", [128, 16384], f32)      # gathers->wih->wfc
    xs_T = nc.alloc_sbuf_tensor("xs_T", [128, NK * T], f32)         # 32KB/part
    A_sb = nc.alloc_sbuf_tensor("A_sb", [128, NJ * T], f32)         # 16KB/part
    id_sb = nc.alloc_sbuf_tensor("id_sb", [128, 128], f32)
    ones_p = nc.alloc_sbuf_tensor("ones_p", [128, 1], f32)
    ones_f = nc.alloc_sbuf_tensor("ones_f", [1, 128], f32)
    b_sb = nc.alloc_sbuf_tensor("b_sb", [128, NJ], f32)
    bfc_sb = nc.alloc_sbuf_tensor("bfc_sb", [128, 4], f32)
    idxa_sb = nc.alloc_sbuf_tensor("idxa_sb", [128, 4], i32)
    idxb_sb = nc.alloc_sbuf_tensor("idxb_sb", [128, 4], i32)
    h_all_sb = nc.alloc_sbuf_tensor("h_all_sb", [128, 32], bf16)    # 2 parity halves
    h_new_sb = nc.alloc_sbuf_tensor("h_new_sb", [128, 4], bf16)
    gates_sb = nc.alloc_sbuf_tensor("gates_sb", [128, 16], f32)
    nl_sb = nc.alloc_sbuf_tensor("nl_sb", [128, 16], f32)
    tmp_sb = nc.alloc_sbuf_tensor("tmp_sb", [128, 8], f32)
    cq_sb = nc.alloc_sbuf_tensor("cq_sb", [128, 4], f32)
    tc_sb = nc.alloc_sbuf_tensor("tc_sb", [128, 4], f32)
    fcl_sb = nc.alloc_sbuf_tensor("fcl_sb", [128, 4], f32)
    exp_sb = nc.alloc_sbuf_tensor("exp_sb", [128, 4], f32)
    esm_sb = nc.alloc_sbuf_tensor("esm_sb", [128, 32], f32)
    osb = nc.alloc_sbuf_tensor("osb", [128, 32], f32)
    red_sb = nc.alloc_sbuf_tensor("red_sb", [128, 1], f32)
    rs_sb = nc.alloc_sbuf_tensor("rs_sb", [1, 1], f32)

    psum = [nc.alloc_psum_tensor(f"ps{j}", [128, 512], f32) for j in range(8)]

    cores = list(range(M))

    # --- static semaphore schedule ---------------------------------------
    PE_TRANS = 64                      # after 64 transposes
    PE_APRE = lambda j: PE_TRANS + j + 1      # after A-precompute col-block j
    PE_STEP = lambda t: 72 + t                # after step-t matvecs (t>=1)
    PE_FC = 584
    PE_SUM = 585
    PE_BC = 586
    G_GATH = 128                               # 8 gathers done
    G_OUT = lambda t: G_GATH + 32 * t + 16     # h_loc out-dma of step t
    G_IN = lambda t: G_GATH + 32 * t + 32      # h_all in-dma of step t
    G_ELOC = G_IN(T - 1) + 4 * 16
    G_ESM = G_ELOC + 32 * 16
    DV_A = 8
    DV_C0 = 9
    DV_H0 = 10
    DV_GATES = lambda t: 3 * t + 8
    DV_C = lambda t: 3 * t + 9
    DV_H = lambda t: 3 * t + 10
    DV_FC = DV_H(T - 1) + 1            # 1544
    DV_RED = DV_FC + 1
    DV_RECIP = DV_FC + 2
    DV_OUT = DV_FC + 3
    AC_COPY = lambda i: i + 1
    AC_A1 = lambda t: 2 * t + 65
    AC_A2 = lambda t: 2 * t + 66
    AC_EXP = AC_A2(T - 1) + 1          # 1089
    LD_WHH, LD_BIAS, LD_BFC, LD_WFC, LD_WIH = 16, 32, 48, 64, 80
    LDI_IDX = 32

    with (
        nc.Block() as block,
        nc.semaphore("ld") as ld,
        nc.semaphore("ldi") as ldi,
        nc.semaphore("gc") as gc,
        nc.semaphore("g16") as g16,
        nc.semaphore("cc") as cc,
        nc.semaphore("pe") as pe,
        nc.semaphore("dv") as dv,
        nc.semaphore("ac") as ac,
        nc.semaphore("vw") as vw,
    ):

        @block.sync
        def _(eng):
            # (p, c) = idx[4p + c]: gather tile c holds timestep t = 4p + c at partition p
            eng.dma_start(out=idxa_sb[:, :], in_=AP(idx_a, 0, [[4, 128], [1, 4]])).then_inc(ldi, 16)
            eng.dma_start(out=idxb_sb[:, :], in_=AP(idx_b, 0, [[4, 128], [1, 4]])).then_inc(ldi, 16)
            eng.dma_start(
                out=AP(whh_sb, 0, [[16384, 128], [1024, 16], [1, 1024]]),
                in_=AP(whh_t, 0, [[1024, 128], [131072, 16], [1, 1024]]),
            ).then_inc(ld, 16)
            eng.dma_start(out=b_sb[:, :], in_=AP(bias_d, 0, [[8, 128], [1, 8]])).then_inc(ld, 16)
            eng.dma_start(out=bfc_sb[:, :], in_=AP(bfc_d, 0, [[4, 128], [1, 4]])).then_inc(ld, 16)
            eng.dma_start(
                out=AP(wfc_sb, 0, [[8192, 128], [512, 16], [1, 512]]),
                in_=AP(wfc_t, 0, [[512, 128], [65536, 16], [1, 512]]),
            ).then_inc(ld, 16)
            eng.wait_ge(pe, PE_TRANS)      # transposes done reading big_sb
            eng.dma_start(
                out=AP(big_sb, 0, [[16384, 128], [1024, 16], [1, 1024]]),
                in_=AP(wih_t, 0, [[1024, 128], [131072, 16], [1, 1024]]),
            ).then_inc(ld, 16)

        @block.gpsimd
        def _(eng):
            eng.memset(id_sb[:, :], 1.0).then_inc(gc, 1)
            eng.memset(ones_p[:, :], 1.0).then_inc(gc, 1)
            eng.memset(ones_f[:, :], 1.0).then_inc(gc, 1)
            eng.wait_ge(gc, 3)
            eng.affine_select(
                id_sb[:, :], id_sb[:, :], pattern=[[1, 128]],
                compare_op=mybir.AluOpType.is_equal, fill=0.0,
                base=0, channel_multiplier=-1,
            ).then_inc(gc, 1)
            eng.wait_ge(ldi, LDI_IDX)
            for c in range(4):
                eng.indirect_dma_start(
                    out=big_sb[:, 1024 * c:1024 * c + 1024], out_offset=None,
                    in_=tab_a[:, :],
                    in_offset=bass.IndirectOffsetOnAxis(ap=idxa_sb[:, c:c + 1], axis=0),
                ).then_inc(g16, 16)
            for c in range(4):
                eng.indirect_dma_start(
                    out=big_sb[:, 4096 + 1024 * c:4096 + 1024 * c + 1024], out_offset=None,
                    in_=tab_b[:, :],
                    in_offset=bass.IndirectOffsetOnAxis(ap=idxb_sb[:, c:c + 1], axis=0),
                ).then_inc(g16, 16)
            for t in range(T):
                tq = t % 2
                eng.wait_ge(dv, DV_H0 if t == 0 else DV_H(t))
                eng.dma_start(
                    out=AP(h_loc, 0, [[2, 128], [1, 2]]),
                    in_=h_new_sb[:, 2 * tq:2 * tq + 2],
                ).then_inc(g16, 16)
                eng.wait_ge(g16, G_OUT(t))
                eng.collective_compute(
                    "AllGather", mybir.AluOpType.bypass,
                    replica_groups=[cores],
                    ins=[h_loc[:]], outs=[h_all[:]],
                ).then_inc(cc, 1)
                eng.wait_ge(cc, t + 1)
                eng.dma_start(
                    out=h_all_sb[:, 16 * tq:16 * tq + 16],
                    in_=AP(h_all, 0, [[16, 128], [1, 16]]),
                ).then_inc(g16, 16)
            eng.wait_ge(ac, AC_EXP)
            for j in range(4):
                eng.dma_start(
                    out=AP(e_loc, 128 * j, [[1, 128], [1, 1]]),
                    in_=exp_sb[:, j:j + 1],
                ).then_inc(g16, 16)
            eng.wait_ge(g16, G_ELOC)
            eng.collective_compute(
                "AllGather", mybir.AluOpType.bypass,
                replica_groups=[cores],
                ins=[e_loc[:]], outs=[e_all[:]],
            ).then_inc(cc, 1)
            eng.wait_ge(cc, T + 1)
            for c in range(32):
                eng.dma_start(
                    out=esm_sb[:, c:c + 1],
                    in_=AP(e_all, 128 * c, [[1, 128], [1, 1]]),
                ).then_inc(g16, 16)
            eng.wait_ge(dv, DV_OUT)
            for c in range(32):
                eng.dma_start(
                    out=AP(out_ext, 128 * c, [[1, 128], [1, 1]]),
                    in_=osb[:, c:c + 1],
                ).then_inc(g16, 16)

        @block.tensor
        def _(eng):
            eng.wait_ge(gc, 4)
            eng.wait_ge(g16, G_GATH)
            for i in range(64):                      # i = 16*cp + k
                cp, k = i // 16, i % 16
                if i >= 8:
                    eng.wait_ge(ac, AC_COPY(i - 8))  # bank free after copy
                src_col = (1024 * cp + 128 * k) if k < 8 else (4096 + 1024 * cp + 128 * (k - 8))
                eng.transpose(
                    psum[i % 8][:, 0:128],
                    big_sb[:, src_col:src_col + 128],
                    id_sb[:, :],
                ).then_inc(pe, 1)
            eng.wait_ge(ld, LD_WIH)
            eng.wait_ge(ac, AC_COPY(63))             # xs_T fully written
            for j in range(NJ):
                for c in range(NK):
                    inst = eng.matmul(
                        psum[j][:, 0:512],
                        big_sb[:, 1024 * c + 128 * j:1024 * c + 128 * j + 128],
                        xs_T[:, 512 * c:512 * c + 512],
                        start=(c == 0), stop=(c == NK - 1),
                    )
                    if c == NK - 1:
                        inst.then_inc(pe, 1)
            eng.wait_ge(dv, DV_A)                    # A-adds done: psum 0/1 free
            for t in range(1, T):
                tq, rq = t % 2, (t - 1) % 2
                eng.wait_ge(g16, G_IN(t - 1))
                for j in range(NJ):
                    for k in range(NK):
                        inst = eng.matmul(
                            psum[tq][:, j:j + 1],
                            whh_sb[:, 1024 * k + 128 * j:1024 * k + 128 * j + 128],
                            h_all_sb[:, 16 * rq + k:16 * rq + k + 1],
                            start=(k == 0), stop=(k == NK - 1),
                        )
                        if j == NJ - 1 and k == NK - 1:
                            inst.then_inc(pe, 1)
            eng.wait_ge(g16, G_IN(T - 1))
            eng.wait_ge(ld, LD_WFC)
            fq = (T - 1) % 2
            for j in range(4):
                for k in range(NK):
                    inst = eng.matmul(
                        psum[2][:, j:j + 1],
                        wfc_sb[:, 512 * k + 128 * j:512 * k + 128 * j + 128],
                        h_all_sb[:, 16 * fq + k:16 * fq + k + 1],
                        start=(k == 0), stop=(k == NK - 1),
                    )
                    if j == 3 and k == NK - 1:
                        inst.then_inc(pe, 1)
            eng.wait_ge(dv, DV_RED)
            eng.matmul(psum[2][0:1, 8:9], ones_p[:, :], red_sb[:, :],
                       start=True, stop=True).then_inc(pe, 1)
            eng.wait_ge(dv, DV_RECIP)
            eng.matmul(psum[2][:, 9:10], ones_f[:, :], rs_sb[:, :],
                       start=True, stop=True).then_inc(pe, 1)

        @block.vector
        def _(eng):
            for j in range(NJ):
                eng.wait_ge(pe, PE_APRE(j))
                eng.tensor_scalar_add(
                    AP(A_sb, j, [[NJ * T, 128], [NJ, T]]),
                    psum[j][:, 0:512],
                    b_sb[:, j:j + 1],
                ).then_inc(dv, 1)
            eng.wait_ge(dv, DV_A)          # A_sb writes retired before self-reads
            # step 0: gates come straight from A (h=0), c_in=0 -> c = sig(i)*tanh(g)
            eng.wait_ge(ac, AC_A1(0))
            eng.tensor_mul(cq_sb[:, 0:2], nl_sb[:, 0:2], nl_sb[:, 4:6]).then_inc(dv, 1)
            eng.wait_ge(ac, AC_A2(0))
            eng.tensor_mul(h_new_sb[:, 0:2], nl_sb[:, 6:8], tc_sb[:, 0:2]).then_inc(dv, 1)
            for t in range(1, T):
                tq = t % 2
                eng.wait_ge(pe, PE_STEP(t))
                eng.tensor_add(
                    gates_sb[:, 8 * tq:8 * tq + 8],
                    psum[tq][:, 0:8],
                    A_sb[:, 8 * t:8 * t + 8],
                ).then_inc(dv, 1)
                eng.wait_ge(ac, AC_A1(t))
                eng.tensor_mul(
                    tmp_sb[:, 4 * tq:4 * tq + 2],
                    nl_sb[:, 8 * tq + 2:8 * tq + 4],
                    AP(xs_T, t, [[NK * T, 128], [512, 2]]),   # c_in = x_t (features 0:256)
                ).then_inc(vw, 1)
                eng.tensor_mul(
                    tmp_sb[:, 4 * tq + 2:4 * tq + 4],
                    nl_sb[:, 8 * tq:8 * tq + 2],
                    nl_sb[:, 8 * tq + 4:8 * tq + 6],
                ).then_inc(vw, 1)
                eng.wait_ge(vw, 2 * t)
                eng.tensor_add(
                    cq_sb[:, 2 * tq:2 * tq + 2],
                    tmp_sb[:, 4 * tq:4 * tq + 2],
                    tmp_sb[:, 4 * tq + 2:4 * tq + 4],
                ).then_inc(dv, 1)
                eng.wait_ge(ac, AC_A2(t))
                eng.tensor_mul(
                    h_new_sb[:, 2 * tq:2 * tq + 2],
                    nl_sb[:, 8 * tq + 6:8 * tq + 8],
                    tc_sb[:, 2 * tq:2 * tq + 2],
                ).then_inc(dv, 1)
            eng.wait_ge(pe, PE_FC)
            eng.tensor_add(fcl_sb[:, :], psum[2][:, 0:4], bfc_sb[:, :]).then_inc(dv, 1)
            eng.wait_ge(g16, G_ESM)
            eng.tensor_reduce(red_sb[:, :], esm_sb[:, :],
                              axis=mybir.AxisListType.X, op=mybir.AluOpType.add).then_inc(dv, 1)
            eng.wait_ge(pe, PE_SUM)
            eng.reciprocal(rs_sb[:, :], psum[2][0:1, 8:9]).then_inc(dv, 1)
            eng.wait_ge(pe, PE_BC)
            eng.tensor_scalar_mul(osb[:, :], esm_sb[:, :], psum[2][:, 9:10]).then_inc(dv, 1)

        @block.scalar
        def _(eng):
            for i in range(64):
                cp, k = i // 16, i % 16
                eng.wait_ge(pe, i + 1)
                # transpose out free index i maps to t = 4i + cp -> stride-4 scatter
                eng.activation(
                    AP(xs_T, 512 * k + cp, [[NK * T, 128], [4, 128]]),
                    psum[i % 8][:, 0:128], Cpy,
                ).then_inc(ac, 1)
            for t in range(T):
                tq = t % 2
                if t == 0:
                    eng.wait_ge(dv, DV_A)
                    g_ap = A_sb
                    base = 0
                else:
                    eng.wait_ge(dv, DV_GATES(t))
                    g_ap = gates_sb
                    base = 8 * tq
                eng.activation(nl_sb[:, 8 * tq:8 * tq + 4], g_ap[:, base:base + 4], Sig)
                eng.activation(nl_sb[:, 8 * tq + 4:8 * tq + 6], g_ap[:, base + 4:base + 6], Tnh)
                eng.activation(nl_sb[:, 8 * tq + 6:8 * tq + 8], g_ap[:, base + 6:base + 8], Sig).then_inc(ac, 1)
                eng.wait_ge(dv, DV_C0 if t == 0 else DV_C(t))
                eng.activation(tc_sb[:, 2 * tq:2 * tq + 2], cq_sb[:, 2 * tq:2 * tq + 2], Tnh).then_inc(ac, 1)
            eng.wait_ge(dv, DV_FC)
            eng.activation(exp_sb[:, :], fcl_sb[:, :], ExpF).then_inc(ac, 1)

    return nc


LAST_EXEC_NS = None


def kernel(**inputs):
    global LAST_EXEC_NS
    from concourse import bass_utils

    if "nc" not in _CACHE:
        _CACHE["nc"] = _build()
    nc = _CACHE["nc"]

    in_maps = _prep_in_maps(inputs)
    trace = bool(int(os.environ.get("KERNEL_TRACE", "0")))
    if trace:
        try:
            res = bass_utils.run_bass_kernel_spmd(nc, in_maps, list(range(M)), trace=True)
        except Exception:
            res = bass_utils.run_bass_kernel_spmd(nc, in_maps, list(range(M)), trace=False)
    else:
        res = bass_utils.run_bass_kernel_spmd(nc, in_maps, list(range(M)), trace=False)
    LAST_EXEC_NS = getattr(res, "exec_time_ns", None)
    out = np.asarray(res.results[0]["out"], dtype=np.float32)
    return out.reshape(1, V)
